# revision 17
# baseline (speedup 1.0000x reference)
"""Mamba BasicBlock kernel for 8 Trainium2 NeuronCores.

Sharding: 2 batches x 4 channel-slices (D_INNER 1536 -> 4 slices of 384).
Core c = b*4 + j handles batch b, channels [j*384,(j+1)*384), full L=2048.
Cross-core comms: 4 token-chunked AllReduces of the x_proj partial
([80,512] each) and ReduceScatter of out_proj partials (each core ends
up with a 512-token slice of the final hidden states).

The whole layer is pipelined over 512-token blocks: for each block nb,
in_proj -> conv -> x_proj -> AllReduce -> delta -> selective scan ->
gate/finalize -> out_proj columns all run inside one loop iteration, so
the PE/Act/Pool/DMA work of block nb+1 overlaps the DVE-bound scan of
block nb. Engine assignment (DVE is the bottleneck):
  LN stats on Act (Copy/Square + accum_out), tiny LN scalars on DVE,
  LN apply on Act, xn transposed via DMA xbar transpose; in_proj /
  x_proj / dt / out_proj matmuls in bf16 on PE; conv = 4 DVE bf16
  tensor_scalar muls + 3 Pool adds + Act SiLU; B/C rows replicated
  across partitions by DMA broadcast from DRAM; dA = Exp(A_n*delta) on
  Act; dBx = (delta*xc) x B on Pool (bf16 TensorTensor); the 16
  tensor_tensor_scans per (block, d-tile) on DVE (fp32 carried state);
  y = sum_n h*C via DVE bf16 mul + tree-adds; +D_skip*xc, *silu(z) on
  DVE; ReduceScatter + per-m output DMA tail.
"""

import os
import sys

sys.path.insert(0, "/opt/trn_rl_repo")

import numpy as np
import ml_dtypes
from contextlib import ExitStack

import concourse.bass as bass
import concourse.bacc as bacc
import concourse.mybir as mybir
import concourse.tile as tile
from concourse.bass_utils import run_bass_kernel_spmd

F = mybir.dt.float32
BF = mybir.dt.bfloat16
FR = mybir.dt.float32r
AF = mybir.ActivationFunctionType
OP = mybir.AluOpType

B, L, DM = 2, 2048, 768
DI, DS, DC, DTR = 1536, 16, 4, 48
SL = 384          # channel slice per core
NJ = 3            # d-tiles of 128 per core
KT = DM // 128    # 6 contraction tiles for in_proj
TB = 512          # token block
NBLK = L // TB
NH = DS // 2      # states per half
NT = L // 128     # LN token tiles
NCORES = 8
GROUPS = [[0, 1, 2, 3], [4, 5, 6, 7]]
LN_EPS = 1e-5
TOK = L // 4      # token slice per core for outputs

_CACHE = {}


def _build(single=False):
    key = "nc1" if single else "nc"
    if key in _CACHE:
        return _CACHE[key]

    nc = bacc.Bacc("TRN2", target_bir_lowering=False, debug=False,
                   num_devices=1 if single else NCORES)

    # ---------------- I/O ----------------
    x_b = nc.dram_tensor("x_b", [L, DM], F, kind="ExternalInput").ap()
    res_x = nc.dram_tensor("res_x", [TOK, DM], F, kind="ExternalInput").ap()
    res_in = nc.dram_tensor("res_in", [TOK, DM], F, kind="ExternalInput").ap()
    W_inT = nc.dram_tensor("W_inT", [DM, 2 * SL], BF, kind="ExternalInput").ap()
    bias_in = nc.dram_tensor("bias_in", [2 * SL], F, kind="ExternalInput").ap()
    WxT = nc.dram_tensor("WxT", [SL, 80], BF, kind="ExternalInput").ap()
    WdtT = nc.dram_tensor("WdtT", [DTR, SL], F, kind="ExternalInput").ap()
    bdt = nc.dram_tensor("bdt", [SL], F, kind="ExternalInput").ap()
    Acols = nc.dram_tensor("Acols", [SL, DS], F, kind="ExternalInput").ap()
    convw = nc.dram_tensor("convw", [SL, DC], F, kind="ExternalInput").ap()
    convb = nc.dram_tensor("convb", [SL], F, kind="ExternalInput").ap()
    Dskip = nc.dram_tensor("Dskip", [SL], F, kind="ExternalInput").ap()
    WoT = nc.dram_tensor("WoT", [SL, DM], BF, kind="ExternalInput").ap()
    hid_out = nc.dram_tensor("hid_out", [DM, TOK], F, kind="ExternalOutput").ap()
    res_out = nc.dram_tensor("res_out", [TOK, DM], F, kind="ExternalOutput").ap()

    with tile.TileContext(nc, trace_sim=False) as tc, ExitStack() as top:
        dram = top.enter_context(tc.tile_pool(name="dram", bufs=1, space="DRAM"))
        proj_part = dram.tile([4 * 80, TB], F)
        proj_sum = dram.tile([4 * 80, TB], F)
        bc_bf = dram.tile([2 * DS, L], BF)  # B/C rows in bf16
        op_part = dram.tile([4 * DM, TOK], F)
        op_rs = dram.tile([DM, TOK], F)

        const = top.enter_context(tc.tile_pool(name="const", bufs=1))
        bias_sb = const.tile([128, 6], F)     # col m: bias_in[m*128+p]
        nc.sync.dma_start(bias_sb[:], bias_in.rearrange("(m p) -> p m", p=128))
        acol_sb = const.tile([128, NJ * DS], F)  # col j*16+n: A[j*128+p, n]
        nc.sync.dma_start(acol_sb[:].rearrange("p (j n) -> p j n", j=NJ),
                          Acols.rearrange("(j p) n -> p j n", p=128))
        convw_sb = const.tile([128, NJ * DC], F)
        nc.sync.dma_start(convw_sb[:].rearrange("p (j k) -> p j k", j=NJ),
                          convw.rearrange("(j p) k -> p j k", p=128))
        convb_sb = const.tile([128, NJ], F)
        nc.sync.dma_start(convb_sb[:], convb.rearrange("(j p) -> p j", p=128))
        dskip_sb = const.tile([128, NJ], F)
        nc.sync.dma_start(dskip_sb[:], Dskip.rearrange("(j p) -> p j", p=128))
        bdt_sb = const.tile([128, NJ], F)
        nc.sync.dma_start(bdt_sb[:], bdt.rearrange("(j p) -> p j", p=128))

        persist = top.enter_context(tc.tile_pool(name="persist", bufs=1))
        carry = [persist.tile([128, DS], F, tag=f"cr{j}", name=f"cr{j}")
                 for j in range(NJ)]
        for j in range(NJ):
            nc.gpsimd.memset(carry[j][:], 0.0)
        x_pad = [persist.tile([128, L + DC - 1], BF, tag=f"xp{j}", name=f"xp{j}")
                 for j in range(NJ)]
        for j in range(NJ):
            nc.gpsimd.memset(x_pad[j][:, 0:DC - 1], 0.0)
        xnT_all = persist.tile([128, KT * L], BF)
        winT_sb = [persist.tile([128, 2 * SL], BF, tag=f"wi{k}", name=f"wi{k}")
                   for k in range(KT)]
        for k in range(KT):
            nc.sync.dma_start(winT_sb[k][:], W_inT[bass.ts(k, 128), :])
        wxT_sb = [persist.tile([128, 80], BF, tag=f"wx{j}", name=f"wx{j}")
                  for j in range(NJ)]
        for j in range(NJ):
            nc.sync.dma_start(wxT_sb[j][:], WxT[bass.ts(j, 128), :])
        wdtT_sb = persist.tile([DTR, SL], FR, tag="wdt")
        nc.sync.dma_start(wdtT_sb[:], WdtT.bitcast(FR))
        woT_sb = [persist.tile([128, DM], BF, tag=f"wo{j}", name=f"wo{j}")
                  for j in range(NJ)]
        for j in range(NJ):
            nc.sync.dma_start(woT_sb[j][:], WoT[bass.ts(j, 128), :])

        # ============ LayerNorm + transpose (two-pass pipeline) ============
        with tc.tile_pool(name="lnp", bufs=1) as lpp, \
             tc.tile_pool(name="ln", bufs=3) as lp:
            scr1 = lpp.tile([128, DM], BF, tag="scr1")
            scr2 = lpp.tile([128, DM], BF, tag="scr2")
            xts, sus, sqs = [], [], []
            for tt in range(NT):
                xt = lpp.tile([128, DM], F, tag=f"xt{tt}", name=f"xt{tt}")
                nc.sync.dma_start(xt[:], x_b[bass.ts(tt, 128), :])
                su = lpp.tile([128, 1], F, tag=f"su{tt}", name=f"su{tt}")
                nc.scalar.activation(scr1[:], xt[:], AF.Copy, accum_out=su[:])
                sq = lpp.tile([128, 1], F, tag=f"sq{tt}", name=f"sq{tt}")
                nc.scalar.activation(scr2[:], xt[:], AF.Square, accum_out=sq[:])
                xts.append(xt); sus.append(su); sqs.append(sq)
            for tt in range(NT):
                mu = lp.tile([128, 1], F, tag="mu")
                nc.vector.tensor_scalar(out=mu[:], in0=sus[tt][:],
                                        scalar1=1.0 / DM, scalar2=None,
                                        op0=OP.mult)
                ex = lp.tile([128, 1], F, tag="ex")
                nc.vector.tensor_scalar(out=ex[:], in0=sqs[tt][:],
                                        scalar1=1.0 / DM, scalar2=None,
                                        op0=OP.mult)
                mu2 = lp.tile([128, 1], F, tag="mu2")
                nc.vector.tensor_scalar(out=mu2[:], in0=mu[:],
                                        scalar1=mu[:, 0:1], scalar2=-LN_EPS,
                                        op0=OP.mult, op1=OP.add)
                var = lp.tile([128, 1], F, tag="var")
                nc.vector.tensor_tensor(out=var[:], in0=ex[:], in1=mu2[:],
                                        op=OP.subtract)
                sd = lp.tile([128, 1], F, tag="sd")
                nc.scalar.activation(sd[:], var[:], AF.Sqrt)
                rstd = lp.tile([128, 1], F, tag="rstd")
                nc.vector.reciprocal(rstd[:], sd[:])
                nmr = lp.tile([128, 1], F, tag="nmr")
                nc.vector.tensor_scalar(out=nmr[:], in0=mu[:],
                                        scalar1=rstd[:, 0:1], scalar2=-1.0,
                                        op0=OP.mult, op1=OP.mult)
                xn = lp.tile([128, DM], BF, tag="xn")
                nc.scalar.activation(xn[:], xts[tt][:], AF.Identity,
                                     scale=rstd[:, 0:1], bias=nmr[:, 0:1])
                nc.sync.dma_start_transpose(
                    xnT_all[:].rearrange("p (k t) -> p k t", k=KT)
                    [:, :, bass.ts(tt, 128)], xn[:])

        # ---- residual output (independent; fills early DMA/Pool idle) ----
        with tc.tile_pool(name="res", bufs=2) as rp:
            for t4 in range(TOK // 128):
                rx = rp.tile([128, DM], F)
                rr = rp.tile([128, DM], F)
                ro = rp.tile([128, DM], F)
                nc.sync.dma_start(rx[:], res_x[bass.ts(t4, 128), :])
                nc.sync.dma_start(rr[:], res_in[bass.ts(t4, 128), :])
                nc.gpsimd.tensor_tensor(out=ro[:], in0=rx[:], in1=rr[:],
                                        op=OP.add)
                nc.sync.dma_start(res_out[bass.ts(t4, 128), :], ro[:])

        # ============ main per-block pipeline ============
        mb = ExitStack()
        with mb:
            bkp = mb.enter_context(tc.tile_pool(name="blk", bufs=2))
            dtp = mb.enter_context(tc.tile_pool(name="dtb", bufs=1))
            nbp = mb.enter_context(tc.tile_pool(name="nbtmp", bufs=1))
            cp = mb.enter_context(tc.tile_pool(name="conv", bufs=1))
            ipp = mb.enter_context(tc.tile_pool(name="ippsum", bufs=2, space="PSUM"))
            xps = mb.enter_context(tc.tile_pool(name="xpps", bufs=1, space="PSUM"))
            dps = mb.enter_context(tc.tile_pool(name="dtps", bufs=2, space="PSUM"))
            ops = mb.enter_context(tc.tile_pool(name="opps", bufs=2, space="PSUM"))
            bp = mb.enter_context(tc.tile_pool(name="brep", bufs=2))
            adp = mb.enter_context(tc.tile_pool(name="sdA", bufs=1))
            dbp = mb.enter_context(tc.tile_pool(name="sdbx", bufs=2))
            ypp = mb.enter_context(tc.tile_pool(name="syp", bufs=1))
            hp = mb.enter_context(tc.tile_pool(name="sh", bufs=2))
            up = mb.enter_context(tc.tile_pool(name="su", bufs=2))
            ytp = mb.enter_context(tc.tile_pool(name="syt", bufs=1))
            fp = mb.enter_context(tc.tile_pool(name="fin", bufs=1))
            op_ = mb.enter_context(tc.tile_pool(name="oproj", bufs=2))
            xc_t, z_t, dl_t, y_t, ub_t = {}, {}, {}, {}, {}

            def emit_outproj(nb):
                for m in range(6):
                    ps = ops.tile([128, TB], F)
                    for j in range(NJ):
                        nc.tensor.matmul(ps[:], woT_sb[j][:, bass.ts(m, 128)],
                                         y_t[j][:],
                                         start=(j == 0), stop=(j == NJ - 1))
                    otn = op_.tile([128, TB], F, tag="otn")
                    nc.scalar.copy(otn[:], ps[:])
                    nc.sync.dma_start(
                        op_part[(m * 4 + nb) * 128:(m * 4 + nb + 1) * 128, :],
                        otn[:])

            for nb in range(4):
                tsl = slice(nb * TB, (nb + 1) * TB)
                # ---- in_proj for this token block ----
                for m in range(6):
                    ps = ipp.tile([128, TB], F)
                    for k in range(KT):
                        nc.tensor.matmul(ps[:],
                                         winT_sb[k][:, bass.ts(m, 128)],
                                         xnT_all[:, k * L + nb * TB:
                                                 k * L + (nb + 1) * TB],
                                         start=(k == 0), stop=(k == KT - 1))
                    if m < 3:
                        nc.scalar.activation(
                            x_pad[m][:, DC - 1 + nb * TB:
                                     DC - 1 + (nb + 1) * TB],
                            ps[:], AF.Identity, bias=bias_sb[:, m:m + 1])
                    else:
                        z = bkp.tile([128, TB], BF, tag=f"z{m - 3}",
                                     name=f"z{m - 3}")
                        nc.scalar.activation(z[:], ps[:], AF.Silu,
                                             bias=bias_sb[:, m:m + 1])
                        z_t[m - 3] = z
                # ---- depthwise conv chunk + SiLU ----
                for j in range(NJ):
                    terms = []
                    for k in range(DC):
                        ak = cp.tile([128, TB], BF, tag=f"cm{k}", name=f"cm{k}")
                        nc.vector.tensor_scalar(
                            out=ak[:], in0=x_pad[j][:, nb * TB + k:
                                                    nb * TB + k + TB],
                            scalar1=convw_sb[:, j * DC + k:j * DC + k + 1],
                            scalar2=None, op0=OP.mult)
                        terms.append(ak)
                    s01 = cp.tile([128, TB], BF, tag="s01")
                    nc.gpsimd.tensor_tensor(out=s01[:], in0=terms[0][:],
                                            in1=terms[1][:], op=OP.add)
                    s23 = cp.tile([128, TB], BF, tag="s23")
                    nc.gpsimd.tensor_tensor(out=s23[:], in0=terms[2][:],
                                            in1=terms[3][:], op=OP.add)
                    st = cp.tile([128, TB], BF, tag="st")
                    nc.gpsimd.tensor_tensor(out=st[:], in0=s01[:],
                                            in1=s23[:], op=OP.add)
                    xc = bkp.tile([128, TB], BF, tag=f"xc{j}", name=f"xc{j}")
                    nc.scalar.activation(xc[:], st[:], AF.Silu,
                                         bias=convb_sb[:, j:j + 1])
                    xc_t[j] = xc
                # ---- x_proj partial + AllReduce ----
                ps = xps.tile([80, TB], F)
                for j in range(NJ):
                    nc.tensor.matmul(ps[:], wxT_sb[j][:], xc_t[j][:],
                                     start=(j == 0), stop=(j == NJ - 1))
                pp = nbp.tile([80, TB], F, tag="pp")
                nc.scalar.copy(pp[:], ps[:])
                nc.sync.dma_start(proj_part[bass.ts(nb, 80), :], pp[:])
                if single:
                    nc.sync.dma_start(proj_sum[bass.ts(nb, 80), :],
                                      proj_part[bass.ts(nb, 80), :])
                else:
                    nc.gpsimd.collective_compute(
                        "AllReduce", OP.add, replica_groups=GROUPS,
                        ins=[proj_part[nb * 80:(nb + 1) * 80, :].opt()],
                        outs=[proj_sum[nb * 80:(nb + 1) * 80, :].opt()])
                # ---- B/C rows -> bf16 -> DRAM (DMA-broadcast source) ----
                bc32 = nbp.tile([2 * DS, TB], F, tag="bc32")
                nc.sync.dma_start(bc32[:],
                                  proj_sum[nb * 80 + DTR:
                                           nb * 80 + DTR + 2 * DS, :])
                bcb = nbp.tile([2 * DS, TB], BF, tag="bcb")
                nc.scalar.copy(bcb[:], bc32[:])
                nc.sync.dma_start(bc_bf[:, tsl], bcb[:])
                # ---- delta = softplus(W_dt @ dt + b_dt), ub = delta*xc ----
                dtT = dtp.tile([DTR, TB], FR, tag="dtT")
                nc.sync.dma_start(dtT[:],
                                  proj_sum[nb * 80:nb * 80 + DTR, :]
                                  .bitcast(FR))
                for j in range(NJ):
                    psd = dps.tile([128, TB], F)
                    nc.tensor.matmul(psd[:], wdtT_sb[:, bass.ts(j, 128)],
                                     dtT[:], start=True, stop=True)
                    et = nbp.tile([128, TB], F, tag="et")
                    nc.scalar.activation(et[:], psd[:], AF.Exp,
                                         bias=bdt_sb[:, j:j + 1])
                    dl = bkp.tile([128, TB], BF, tag=f"dl{j}", name=f"dl{j}")
                    nc.scalar.activation(dl[:], et[:], AF.Ln, bias=1.0)
                    dl_t[j] = dl
                    ub = up.tile([128, TB], BF, tag=f"ub{j}", name=f"ub{j}")
                    nc.vector.tensor_mul(ub[:], dl[:], xc_t[j][:])
                    ub_t[j] = ub

                # ---- out_proj for the previous block (PE/Act slack) ----
                if nb > 0:
                    emit_outproj(nb - 1)

                # ---- selective scan ----
                for half in range(2):
                    bc_rep = bp.tile([128, 2 * NH * TB], BF, tag="bcr")
                    nc.sync.dma_start(
                        bc_rep[:, 0:NH * TB].rearrange("p (n t) -> p n t",
                                                       n=NH),
                        bc_bf[half * NH:(half + 1) * NH, tsl]
                        .unsqueeze(0).broadcast_to([128, NH, TB]))
                    nc.sync.dma_start(
                        bc_rep[:, NH * TB:2 * NH * TB]
                        .rearrange("p (n t) -> p n t", n=NH),
                        bc_bf[DS + half * NH:DS + (half + 1) * NH, tsl]
                        .unsqueeze(0).broadcast_to([128, NH, TB]))
                    b_rep = bc_rep[:, 0:NH * TB]
                    c_rep = bc_rep[:, NH * TB:2 * NH * TB]

                    for j in range(NJ):
                        dA = adp.tile([128, NH * TB], F, tag="dA")
                        for n in range(NH):
                            nc.scalar.activation(
                                dA[:, bass.ts(n, TB)], dl_t[j][:], AF.Exp,
                                scale=acol_sb[:, j * DS + half * NH + n:
                                              j * DS + half * NH + n + 1])
                        dbx = dbp.tile([128, NH * TB], BF, tag="dbx")
                        for hv in range(2):
                            sl4 = slice(hv * NH // 2 * TB,
                                        (hv + 1) * NH // 2 * TB)
                            nc.gpsimd.tensor_tensor(
                                out=dbx[:, sl4].rearrange(
                                    "p (n t) -> p n t", n=NH // 2),
                                in0=ub_t[j][:].unsqueeze(1)
                                .broadcast_to([128, NH // 2, TB]),
                                in1=b_rep[:, sl4].rearrange(
                                    "p (n t) -> p n t", n=NH // 2),
                                op=OP.mult)
                        h = hp.tile([128, NH * TB], BF, tag="h")
                        for n in range(NH):
                            nc.vector.tensor_tensor_scan(
                                out=h[:, bass.ts(n, TB)],
                                data0=dA[:, bass.ts(n, TB)],
                                data1=dbx[:, bass.ts(n, TB)],
                                initial=carry[j][:, half * NH + n:
                                                 half * NH + n + 1],
                                op0=OP.mult, op1=OP.add)
                        nc.vector.tensor_copy(
                            carry[j][:, half * NH:(half + 1) * NH],
                            h[:].rearrange("p (n t) -> p n t",
                                           n=NH)[:, :, TB - 1])
                        yp = ypp.tile([128, NH * TB], BF, tag="yp")
                        nc.vector.tensor_mul(yp[:], h[:], c_rep)
                        t2 = ytp.tile([128, 4 * TB], BF, tag="t2")
                        nc.vector.tensor_add(t2[:], yp[:, 0:4 * TB],
                                             yp[:, 4 * TB:8 * TB])
                        t3 = ytp.tile([128, 2 * TB], BF, tag="t3")
                        nc.vector.tensor_add(t3[:], t2[:, 0:2 * TB],
                                             t2[:, 2 * TB:4 * TB])
                        if half == 0:
                            y = bkp.tile([128, TB], BF, tag=f"y{j}",
                                         name=f"y{j}")
                            nc.vector.tensor_add(y[:], t3[:, 0:TB],
                                                 t3[:, TB:2 * TB])
                            y_t[j] = y
                        else:
                            yt = ytp.tile([128, TB], BF, tag="yt")
                            nc.vector.tensor_add(yt[:], t3[:, 0:TB],
                                                 t3[:, TB:2 * TB])
                            nc.vector.tensor_add(y_t[j][:], y_t[j][:], yt[:])

                # ---- finalize: y = (D*xc + y) * silu(z) ----
                for j in range(NJ):
                    t0 = fp.tile([128, TB], BF, tag="t0")
                    nc.vector.tensor_scalar(out=t0[:], in0=xc_t[j][:],
                                            scalar1=dskip_sb[:, j:j + 1],
                                            scalar2=None, op0=OP.mult)
                    t1 = fp.tile([128, TB], BF, tag="t1")
                    nc.vector.tensor_add(t1[:], t0[:], y_t[j][:])
                    nc.vector.tensor_mul(y_t[j][:], t1[:], z_t[j][:])

            emit_outproj(3)

            # ---- ReduceScatter + output tail ----
            for m in range(6):
                if single:
                    nc.sync.dma_start(op_rs[bass.ts(m, 128), :],
                                      op_part[m * 512:m * 512 + 128, :])
                else:
                    nc.gpsimd.collective_compute(
                        "ReduceScatter", OP.add, replica_groups=GROUPS,
                        ins=[op_part[m * 512:(m + 1) * 512, :].opt()],
                        outs=[op_rs[bass.ts(m, 128), :].opt()])
                nc.sync.dma_start(hid_out[bass.ts(m, 128), :],
                                  op_rs[bass.ts(m, 128), :])

    nc.compile()
    _CACHE[key] = nc
    return nc


def _prep_inputs(inp):
    gamma, beta = inp["ln_gamma"], inp["ln_beta"]
    W_in = inp["W_in"]
    W_in_f = W_in * gamma[None, :]
    bias_full = W_in @ beta            # [2*DI]
    A = -np.exp(inp["A_log"])          # [DI, DS]
    bf = ml_dtypes.bfloat16

    in_maps = []
    for c in range(NCORES):
        b, j = c // 4, c % 4
        S = slice(j * SL, (j + 1) * SL)
        rows = np.r_[j * SL:(j + 1) * SL, DI + j * SL:DI + (j + 1) * SL]
        m = {
            "x_b": inp["x"][b],
            "res_x": inp["x"][b, j * TOK:(j + 1) * TOK],
            "res_in": inp["residual"][b, j * TOK:(j + 1) * TOK],
            "W_inT": np.ascontiguousarray(W_in_f[rows].T).astype(bf),
            "bias_in": np.ascontiguousarray(bias_full[rows]),
            "WxT": np.ascontiguousarray(inp["W_xproj"][:, S].T).astype(bf),
            "WdtT": np.ascontiguousarray(inp["W_dt"][S].T),
            "bdt": np.ascontiguousarray(inp["b_dt"][S]),
            "Acols": np.ascontiguousarray(A[S]),
            "convw": np.ascontiguousarray(inp["conv_w"][S]),
            "convb": np.ascontiguousarray(inp["conv_b"][S]),
            "Dskip": np.ascontiguousarray(inp["D_skip"][S]),
            "WoT": np.ascontiguousarray(inp["W_out"][:, S].T).astype(bf),
        }
        in_maps.append(m)
    return in_maps


def _assemble(results):
    hidden = np.empty((B, L, DM), np.float32)
    residual = np.empty((B, L, DM), np.float32)
    for c in range(NCORES):
        b, j = c // 4, c % 4
        r = results[c]
        hidden[b, j * TOK:(j + 1) * TOK] = r["hid_out"].T
        residual[b, j * TOK:(j + 1) * TOK] = r["res_out"]
    return hidden, residual


def kernel(**inputs):
    inp = {k: np.ascontiguousarray(np.asarray(v, dtype=np.float32))
           for k, v in inputs.items()}
    nc = _build()
    in_maps = _prep_inputs(inp)
    res = run_bass_kernel_spmd(nc, in_maps, list(range(NCORES)))
    return _assemble(res.results)


# revision 19
# speedup vs baseline: 1.0675x; 1.0675x over previous
"""Mamba BasicBlock kernel for 8 Trainium2 NeuronCores.

Sharding: 2 batches x 4 channel-slices (D_INNER 1536 -> 4 slices of 384).
Core c = b*4 + j handles batch b, channels [j*384,(j+1)*384), full L=2048.
Cross-core comms: 4 token-chunked AllReduces of the x_proj partial
([80,512] each) and ReduceScatter of out_proj partials (each core ends
up with a 512-token slice of the final hidden states).

The whole layer is pipelined over 512-token blocks: for each block nb,
in_proj -> conv -> x_proj -> AllReduce -> delta -> selective scan ->
gate/finalize -> out_proj columns all run inside one loop iteration, so
the PE/Act/Pool/DMA work of block nb+1 overlaps the DVE-bound scan of
block nb. Engine assignment (DVE is the bottleneck):
  LN stats on Act (Copy/Square + accum_out), tiny LN scalars on DVE,
  LN apply on Act, xn transposed via DMA xbar transpose; in_proj /
  x_proj / dt / out_proj matmuls in bf16 on PE; conv = 4 DVE bf16
  tensor_scalar muls + 3 Pool adds + Act SiLU; B/C rows replicated
  across partitions by DMA broadcast from DRAM; dA = Exp(A_n*delta) on
  Act; dBx = (delta*xc) x B on Pool (bf16 TensorTensor); the 16
  tensor_tensor_scans per (block, d-tile) on DVE (fp32 carried state);
  y = sum_n h*C via DVE bf16 mul + tree-adds; +D_skip*xc, *silu(z) on
  DVE; ReduceScatter + per-m output DMA tail.
"""

import os
import sys

sys.path.insert(0, "/opt/trn_rl_repo")

import numpy as np
import ml_dtypes
from contextlib import ExitStack

import concourse.bass as bass
import concourse.bacc as bacc
import concourse.mybir as mybir
import concourse.tile as tile
from concourse.bass_utils import run_bass_kernel_spmd

F = mybir.dt.float32
BF = mybir.dt.bfloat16
FR = mybir.dt.float32r
AF = mybir.ActivationFunctionType
OP = mybir.AluOpType

B, L, DM = 2, 2048, 768
DI, DS, DC, DTR = 1536, 16, 4, 48
SL = 384          # channel slice per core
NJ = 3            # d-tiles of 128 per core
KT = DM // 128    # 6 contraction tiles for in_proj
TB = 512          # token block
NBLK = L // TB
NH = DS // 2      # states per half
NT = L // 128     # LN token tiles
NCORES = 8
GROUPS = [[0, 1, 2, 3], [4, 5, 6, 7]]
LN_EPS = 1e-5
TOK = L // 4      # token slice per core for outputs

_CACHE = {}


def _build(single=False):
    key = "nc1" if single else "nc"
    if key in _CACHE:
        return _CACHE[key]

    nc = bacc.Bacc("TRN2", target_bir_lowering=False, debug=False,
                   num_devices=1 if single else NCORES)

    # ---------------- I/O ----------------
    x_b = nc.dram_tensor("x_b", [L, DM], F, kind="ExternalInput").ap()
    res_x = nc.dram_tensor("res_x", [TOK, DM], F, kind="ExternalInput").ap()
    res_in = nc.dram_tensor("res_in", [TOK, DM], F, kind="ExternalInput").ap()
    W_inT = nc.dram_tensor("W_inT", [DM, 2 * SL], BF, kind="ExternalInput").ap()
    bias_in = nc.dram_tensor("bias_in", [2 * SL], F, kind="ExternalInput").ap()
    WxT = nc.dram_tensor("WxT", [SL, 80], BF, kind="ExternalInput").ap()
    WdtT = nc.dram_tensor("WdtT", [DTR, SL], F, kind="ExternalInput").ap()
    bdt = nc.dram_tensor("bdt", [SL], F, kind="ExternalInput").ap()
    Acols = nc.dram_tensor("Acols", [SL, DS], F, kind="ExternalInput").ap()
    convw = nc.dram_tensor("convw", [SL, DC], F, kind="ExternalInput").ap()
    convb = nc.dram_tensor("convb", [SL], F, kind="ExternalInput").ap()
    Dskip = nc.dram_tensor("Dskip", [SL], F, kind="ExternalInput").ap()
    WoT = nc.dram_tensor("WoT", [SL, DM], BF, kind="ExternalInput").ap()
    hid_out = nc.dram_tensor("hid_out", [DM, TOK], F, kind="ExternalOutput").ap()
    res_out = nc.dram_tensor("res_out", [TOK, DM], F, kind="ExternalOutput").ap()

    with tile.TileContext(nc, trace_sim=False) as tc, ExitStack() as top:
        dram = top.enter_context(tc.tile_pool(name="dram", bufs=1, space="DRAM"))
        proj_part = dram.tile([4 * 80, TB], F)
        proj_sum = dram.tile([4 * 80, TB], F)
        bc_bf = dram.tile([2 * DS, L], BF)  # B/C rows in bf16
        op_part = dram.tile([4 * DM, TOK], F)
        op_rs = dram.tile([DM, TOK], F)

        const = top.enter_context(tc.tile_pool(name="const", bufs=1))
        bias_sb = const.tile([128, 6], F)     # col m: bias_in[m*128+p]
        nc.sync.dma_start(bias_sb[:], bias_in.rearrange("(m p) -> p m", p=128))
        acol_sb = const.tile([128, NJ * DS], F)  # col j*16+n: A[j*128+p, n]
        nc.sync.dma_start(acol_sb[:].rearrange("p (j n) -> p j n", j=NJ),
                          Acols.rearrange("(j p) n -> p j n", p=128))
        convw_sb = const.tile([128, NJ * DC], F)
        nc.sync.dma_start(convw_sb[:].rearrange("p (j k) -> p j k", j=NJ),
                          convw.rearrange("(j p) k -> p j k", p=128))
        convb_sb = const.tile([128, NJ], F)
        nc.sync.dma_start(convb_sb[:], convb.rearrange("(j p) -> p j", p=128))
        dskip_sb = const.tile([128, NJ], F)
        nc.sync.dma_start(dskip_sb[:], Dskip.rearrange("(j p) -> p j", p=128))
        bdt_sb = const.tile([128, NJ], F)
        nc.sync.dma_start(bdt_sb[:], bdt.rearrange("(j p) -> p j", p=128))

        persist = top.enter_context(tc.tile_pool(name="persist", bufs=1))
        carry = [persist.tile([128, DS], F, tag=f"cr{j}", name=f"cr{j}")
                 for j in range(NJ)]
        for j in range(NJ):
            nc.gpsimd.memset(carry[j][:], 0.0)
        x_pad = [persist.tile([128, L + DC - 1], BF, tag=f"xp{j}", name=f"xp{j}")
                 for j in range(NJ)]
        for j in range(NJ):
            nc.gpsimd.memset(x_pad[j][:, 0:DC - 1], 0.0)
        xnT_all = persist.tile([128, KT * L], BF)
        winT_sb = [persist.tile([128, 2 * SL], BF, tag=f"wi{k}", name=f"wi{k}")
                   for k in range(KT)]
        for k in range(KT):
            nc.sync.dma_start(winT_sb[k][:], W_inT[bass.ts(k, 128), :])
        wxT_sb = [persist.tile([128, 80], BF, tag=f"wx{j}", name=f"wx{j}")
                  for j in range(NJ)]
        for j in range(NJ):
            nc.sync.dma_start(wxT_sb[j][:], WxT[bass.ts(j, 128), :])
        wdtT_sb = persist.tile([DTR, SL], FR, tag="wdt")
        nc.sync.dma_start(wdtT_sb[:], WdtT.bitcast(FR))
        woT_sb = [persist.tile([128, DM], BF, tag=f"wo{j}", name=f"wo{j}")
                  for j in range(NJ)]
        for j in range(NJ):
            nc.sync.dma_start(woT_sb[j][:], WoT[bass.ts(j, 128), :])

        # ============ LayerNorm + transpose (two-pass pipeline) ============
        with tc.tile_pool(name="lnp", bufs=1) as lpp, \
             tc.tile_pool(name="ln", bufs=3) as lp:
            xts, mvs, vss = [], [], []
            for tt in range(NT):
                xt = lpp.tile([128, DM], F, tag=f"xt{tt}", name=f"xt{tt}")
                nc.sync.dma_start(xt[:], x_b[bass.ts(tt, 128), :])
                st6 = lp.tile([128, 2 * 6], F, tag="st6")
                nc.vector.bn_stats(st6[:, 0:6], xt[:, 0:DM // 2])
                nc.vector.bn_stats(st6[:, 6:12], xt[:, DM // 2:DM])
                mv = lpp.tile([128, 2], F, tag=f"mv{tt}", name=f"mv{tt}")
                nc.vector.bn_aggr(mv[:], st6[:])
                vs = lpp.tile([128, 1], F, tag=f"vs{tt}", name=f"vs{tt}")
                nc.vector.tensor_scalar(out=vs[:], in0=mv[:, 1:2],
                                        scalar1=1.0, scalar2=LN_EPS,
                                        op0=OP.mult, op1=OP.add)
                xts.append(xt); mvs.append(mv); vss.append(vs)
            sds = []
            for tt in range(NT):
                sd = lpp.tile([128, 1], F, tag=f"sd{tt}", name=f"sd{tt}")
                nc.scalar.activation(sd[:], vss[tt][:], AF.Sqrt)
                sds.append(sd)
            for tt in range(NT):
                rstd = lp.tile([128, 1], F, tag="rstd")
                nc.vector.reciprocal(rstd[:], sds[tt][:])
                nmr = lp.tile([128, 1], F, tag="nmr")
                nc.vector.tensor_scalar(out=nmr[:], in0=mvs[tt][:, 0:1],
                                        scalar1=rstd[:, 0:1], scalar2=-1.0,
                                        op0=OP.mult, op1=OP.mult)
                xn = lp.tile([128, DM], BF, tag="xn")
                nc.vector.tensor_scalar(out=xn[:], in0=xts[tt][:],
                                        scalar1=rstd[:, 0:1],
                                        scalar2=nmr[:, 0:1],
                                        op0=OP.mult, op1=OP.add)
                nc.sync.dma_start_transpose(
                    xnT_all[:].rearrange("p (k t) -> p k t", k=KT)
                    [:, :, bass.ts(tt, 128)], xn[:])

        # ---- residual output (independent; fills early DMA/Pool idle) ----
        with tc.tile_pool(name="res", bufs=2) as rp:
            for t4 in range(TOK // 128):
                rx = rp.tile([128, DM], F)
                rr = rp.tile([128, DM], F)
                ro = rp.tile([128, DM], F)
                nc.sync.dma_start(rx[:], res_x[bass.ts(t4, 128), :])
                nc.sync.dma_start(rr[:], res_in[bass.ts(t4, 128), :])
                nc.gpsimd.tensor_tensor(out=ro[:], in0=rx[:], in1=rr[:],
                                        op=OP.add)
                nc.sync.dma_start(res_out[bass.ts(t4, 128), :], ro[:])

        # ============ main per-block pipeline ============
        mb = ExitStack()
        with mb:
            bkp = mb.enter_context(tc.tile_pool(name="blk", bufs=2))
            dtp = mb.enter_context(tc.tile_pool(name="dtb", bufs=1))
            nbp = mb.enter_context(tc.tile_pool(name="nbtmp", bufs=1))
            cp = mb.enter_context(tc.tile_pool(name="conv", bufs=1))
            ipp = mb.enter_context(tc.tile_pool(name="ippsum", bufs=2, space="PSUM"))
            xps = mb.enter_context(tc.tile_pool(name="xpps", bufs=1, space="PSUM"))
            dps = mb.enter_context(tc.tile_pool(name="dtps", bufs=2, space="PSUM"))
            ops = mb.enter_context(tc.tile_pool(name="opps", bufs=2, space="PSUM"))
            bp = mb.enter_context(tc.tile_pool(name="brep", bufs=2))
            adp = mb.enter_context(tc.tile_pool(name="sdA", bufs=1))
            dbp = mb.enter_context(tc.tile_pool(name="sdbx", bufs=2))
            ypp = mb.enter_context(tc.tile_pool(name="syp", bufs=1))
            hp = mb.enter_context(tc.tile_pool(name="sh", bufs=2))
            up = mb.enter_context(tc.tile_pool(name="su", bufs=2))
            ytp = mb.enter_context(tc.tile_pool(name="syt", bufs=1))
            fp = mb.enter_context(tc.tile_pool(name="fin", bufs=1))
            op_ = mb.enter_context(tc.tile_pool(name="oproj", bufs=2))
            xc_t, z_t, dl_t, y_t, ub_t = {}, {}, {}, {}, {}

            def emit_outproj(nb):
                for m in range(6):
                    ps = ops.tile([128, TB], F)
                    for j in range(NJ):
                        nc.tensor.matmul(ps[:], woT_sb[j][:, bass.ts(m, 128)],
                                         y_t[j][:],
                                         start=(j == 0), stop=(j == NJ - 1))
                    otn = op_.tile([128, TB], F, tag="otn")
                    nc.scalar.copy(otn[:], ps[:])
                    nc.sync.dma_start(
                        op_part[(m * 4 + nb) * 128:(m * 4 + nb + 1) * 128, :],
                        otn[:])

            for nb in range(4):
                tsl = slice(nb * TB, (nb + 1) * TB)
                # ---- in_proj for this token block ----
                for m in range(6):
                    ps = ipp.tile([128, TB], F)
                    for k in range(KT):
                        nc.tensor.matmul(ps[:],
                                         winT_sb[k][:, bass.ts(m, 128)],
                                         xnT_all[:, k * L + nb * TB:
                                                 k * L + (nb + 1) * TB],
                                         start=(k == 0), stop=(k == KT - 1))
                    if m < 3:
                        nc.scalar.activation(
                            x_pad[m][:, DC - 1 + nb * TB:
                                     DC - 1 + (nb + 1) * TB],
                            ps[:], AF.Identity, bias=bias_sb[:, m:m + 1])
                    else:
                        z = bkp.tile([128, TB], BF, tag=f"z{m - 3}",
                                     name=f"z{m - 3}")
                        nc.scalar.activation(z[:], ps[:], AF.Silu,
                                             bias=bias_sb[:, m:m + 1])
                        z_t[m - 3] = z
                # ---- depthwise conv chunk + SiLU ----
                for j in range(NJ):
                    terms = []
                    for k in range(DC):
                        ak = cp.tile([128, TB], BF, tag=f"cm{k}", name=f"cm{k}")
                        nc.vector.tensor_scalar(
                            out=ak[:], in0=x_pad[j][:, nb * TB + k:
                                                    nb * TB + k + TB],
                            scalar1=convw_sb[:, j * DC + k:j * DC + k + 1],
                            scalar2=None, op0=OP.mult)
                        terms.append(ak)
                    s01 = cp.tile([128, TB], BF, tag="s01")
                    nc.gpsimd.tensor_tensor(out=s01[:], in0=terms[0][:],
                                            in1=terms[1][:], op=OP.add)
                    s23 = cp.tile([128, TB], BF, tag="s23")
                    nc.gpsimd.tensor_tensor(out=s23[:], in0=terms[2][:],
                                            in1=terms[3][:], op=OP.add)
                    st = cp.tile([128, TB], BF, tag="st")
                    nc.gpsimd.tensor_tensor(out=st[:], in0=s01[:],
                                            in1=s23[:], op=OP.add)
                    xc = bkp.tile([128, TB], BF, tag=f"xc{j}", name=f"xc{j}")
                    nc.scalar.activation(xc[:], st[:], AF.Silu,
                                         bias=convb_sb[:, j:j + 1])
                    xc_t[j] = xc
                # ---- x_proj partial + AllReduce ----
                ps = xps.tile([80, TB], F)
                for j in range(NJ):
                    nc.tensor.matmul(ps[:], wxT_sb[j][:], xc_t[j][:],
                                     start=(j == 0), stop=(j == NJ - 1))
                pp = nbp.tile([80, TB], F, tag="pp")
                nc.scalar.copy(pp[:], ps[:])
                nc.sync.dma_start(proj_part[bass.ts(nb, 80), :], pp[:])
                if single:
                    nc.sync.dma_start(proj_sum[bass.ts(nb, 80), :],
                                      proj_part[bass.ts(nb, 80), :])
                else:
                    nc.gpsimd.collective_compute(
                        "AllReduce", OP.add, replica_groups=GROUPS,
                        ins=[proj_part[nb * 80:(nb + 1) * 80, :].opt()],
                        outs=[proj_sum[nb * 80:(nb + 1) * 80, :].opt()])
                # ---- B/C rows -> bf16 -> DRAM (DMA-broadcast source) ----
                bc32 = nbp.tile([2 * DS, TB], F, tag="bc32")
                nc.sync.dma_start(bc32[:],
                                  proj_sum[nb * 80 + DTR:
                                           nb * 80 + DTR + 2 * DS, :])
                bcb = nbp.tile([2 * DS, TB], BF, tag="bcb")
                nc.scalar.copy(bcb[:], bc32[:])
                nc.sync.dma_start(bc_bf[:, tsl], bcb[:])
                # ---- delta = softplus(W_dt @ dt + b_dt), ub = delta*xc ----
                dtT = dtp.tile([DTR, TB], FR, tag="dtT")
                nc.sync.dma_start(dtT[:],
                                  proj_sum[nb * 80:nb * 80 + DTR, :]
                                  .bitcast(FR))
                ets = []
                for j in range(NJ):
                    psd = dps.tile([128, TB], F)
                    nc.tensor.matmul(psd[:], wdtT_sb[:, bass.ts(j, 128)],
                                     dtT[:], start=True, stop=True)
                    et = nbp.tile([128, TB], F, tag=f"et{j}", name=f"et{j}")
                    nc.scalar.activation(et[:], psd[:], AF.Exp,
                                         bias=bdt_sb[:, j:j + 1])
                    ets.append(et)
                for j in range(NJ):
                    dl = bkp.tile([128, TB], BF, tag=f"dl{j}", name=f"dl{j}")
                    nc.scalar.activation(dl[:], ets[j][:], AF.Ln, bias=1.0)
                    dl_t[j] = dl
                    ub = up.tile([128, TB], BF, tag=f"ub{j}", name=f"ub{j}")
                    nc.vector.tensor_mul(ub[:], dl[:], xc_t[j][:])
                    ub_t[j] = ub

                # ---- out_proj for the previous block (PE/Act slack) ----
                if nb > 0:
                    emit_outproj(nb - 1)

                # ---- selective scan ----
                for half in range(2):
                    bc_rep = bp.tile([128, 2 * NH * TB], BF, tag="bcr")
                    nc.sync.dma_start(
                        bc_rep[:, 0:NH * TB].rearrange("p (n t) -> p n t",
                                                       n=NH),
                        bc_bf[half * NH:(half + 1) * NH, tsl]
                        .unsqueeze(0).broadcast_to([128, NH, TB]))
                    nc.sync.dma_start(
                        bc_rep[:, NH * TB:2 * NH * TB]
                        .rearrange("p (n t) -> p n t", n=NH),
                        bc_bf[DS + half * NH:DS + (half + 1) * NH, tsl]
                        .unsqueeze(0).broadcast_to([128, NH, TB]))
                    b_rep = bc_rep[:, 0:NH * TB]
                    c_rep = bc_rep[:, NH * TB:2 * NH * TB]

                    for j in range(NJ):
                        dA = adp.tile([128, NH * TB], F, tag="dA")
                        for n in range(NH):
                            nc.scalar.activation(
                                dA[:, bass.ts(n, TB)], dl_t[j][:], AF.Exp,
                                scale=acol_sb[:, j * DS + half * NH + n:
                                              j * DS + half * NH + n + 1])
                        dbx = dbp.tile([128, NH * TB], BF, tag="dbx")
                        for hv in range(2):
                            sl4 = slice(hv * NH // 2 * TB,
                                        (hv + 1) * NH // 2 * TB)
                            nc.gpsimd.tensor_tensor(
                                out=dbx[:, sl4].rearrange(
                                    "p (n t) -> p n t", n=NH // 2),
                                in0=ub_t[j][:].unsqueeze(1)
                                .broadcast_to([128, NH // 2, TB]),
                                in1=b_rep[:, sl4].rearrange(
                                    "p (n t) -> p n t", n=NH // 2),
                                op=OP.mult)
                        h = hp.tile([128, NH * TB], BF, tag="h")
                        for n in range(NH):
                            nc.vector.tensor_tensor_scan(
                                out=h[:, bass.ts(n, TB)],
                                data0=dA[:, bass.ts(n, TB)],
                                data1=dbx[:, bass.ts(n, TB)],
                                initial=carry[j][:, half * NH + n:
                                                 half * NH + n + 1],
                                op0=OP.mult, op1=OP.add)
                        nc.vector.tensor_copy(
                            carry[j][:, half * NH:(half + 1) * NH],
                            h[:].rearrange("p (n t) -> p n t",
                                           n=NH)[:, :, TB - 1])
                        yp = ypp.tile([128, NH * TB], BF, tag="yp")
                        nc.vector.tensor_mul(yp[:], h[:], c_rep)
                        t2 = ytp.tile([128, 4 * TB], BF, tag="t2")
                        nc.vector.tensor_add(t2[:], yp[:, 0:4 * TB],
                                             yp[:, 4 * TB:8 * TB])
                        t3 = ytp.tile([128, 2 * TB], BF, tag="t3")
                        nc.vector.tensor_add(t3[:], t2[:, 0:2 * TB],
                                             t2[:, 2 * TB:4 * TB])
                        if half == 0:
                            y = bkp.tile([128, TB], BF, tag=f"y{j}",
                                         name=f"y{j}")
                            nc.vector.tensor_add(y[:], t3[:, 0:TB],
                                                 t3[:, TB:2 * TB])
                            y_t[j] = y
                        else:
                            yt = ytp.tile([128, TB], BF, tag="yt")
                            nc.vector.tensor_add(yt[:], t3[:, 0:TB],
                                                 t3[:, TB:2 * TB])
                            nc.vector.tensor_add(y_t[j][:], y_t[j][:], yt[:])

                # ---- finalize: y = (D*xc + y) * silu(z) ----
                for j in range(NJ):
                    t0 = fp.tile([128, TB], BF, tag="t0")
                    nc.vector.tensor_scalar(out=t0[:], in0=xc_t[j][:],
                                            scalar1=dskip_sb[:, j:j + 1],
                                            scalar2=None, op0=OP.mult)
                    t1 = fp.tile([128, TB], BF, tag="t1")
                    nc.vector.tensor_add(t1[:], t0[:], y_t[j][:])
                    nc.vector.tensor_mul(y_t[j][:], t1[:], z_t[j][:])

            emit_outproj(3)

            # ---- ReduceScatter + output tail ----
            for m in range(6):
                if single:
                    nc.sync.dma_start(op_rs[bass.ts(m, 128), :],
                                      op_part[m * 512:m * 512 + 128, :])
                else:
                    nc.gpsimd.collective_compute(
                        "ReduceScatter", OP.add, replica_groups=GROUPS,
                        ins=[op_part[m * 512:(m + 1) * 512, :].opt()],
                        outs=[op_rs[bass.ts(m, 128), :].opt()])
                nc.sync.dma_start(hid_out[bass.ts(m, 128), :],
                                  op_rs[bass.ts(m, 128), :])

    nc.compile()
    _CACHE[key] = nc
    return nc


def _prep_inputs(inp):
    gamma, beta = inp["ln_gamma"], inp["ln_beta"]
    W_in = inp["W_in"]
    W_in_f = W_in * gamma[None, :]
    bias_full = W_in @ beta            # [2*DI]
    A = -np.exp(inp["A_log"])          # [DI, DS]
    bf = ml_dtypes.bfloat16

    in_maps = []
    for c in range(NCORES):
        b, j = c // 4, c % 4
        S = slice(j * SL, (j + 1) * SL)
        rows = np.r_[j * SL:(j + 1) * SL, DI + j * SL:DI + (j + 1) * SL]
        m = {
            "x_b": inp["x"][b],
            "res_x": inp["x"][b, j * TOK:(j + 1) * TOK],
            "res_in": inp["residual"][b, j * TOK:(j + 1) * TOK],
            "W_inT": np.ascontiguousarray(W_in_f[rows].T).astype(bf),
            "bias_in": np.ascontiguousarray(bias_full[rows]),
            "WxT": np.ascontiguousarray(inp["W_xproj"][:, S].T).astype(bf),
            "WdtT": np.ascontiguousarray(inp["W_dt"][S].T),
            "bdt": np.ascontiguousarray(inp["b_dt"][S]),
            "Acols": np.ascontiguousarray(A[S]),
            "convw": np.ascontiguousarray(inp["conv_w"][S]),
            "convb": np.ascontiguousarray(inp["conv_b"][S]),
            "Dskip": np.ascontiguousarray(inp["D_skip"][S]),
            "WoT": np.ascontiguousarray(inp["W_out"][:, S].T).astype(bf),
        }
        in_maps.append(m)
    return in_maps


def _assemble(results):
    hidden = np.empty((B, L, DM), np.float32)
    residual = np.empty((B, L, DM), np.float32)
    for c in range(NCORES):
        b, j = c // 4, c % 4
        r = results[c]
        hidden[b, j * TOK:(j + 1) * TOK] = r["hid_out"].T
        residual[b, j * TOK:(j + 1) * TOK] = r["res_out"]
    return hidden, residual


def kernel(**inputs):
    inp = {k: np.ascontiguousarray(np.asarray(v, dtype=np.float32))
           for k, v in inputs.items()}
    nc = _build()
    in_maps = _prep_inputs(inp)
    res = run_bass_kernel_spmd(nc, in_maps, list(range(NCORES)))
    return _assemble(res.results)


# revision 20
# speedup vs baseline: 1.0997x; 1.0301x over previous
"""Mamba BasicBlock kernel for 8 Trainium2 NeuronCores.

Sharding: 2 batches x 4 channel-slices (D_INNER 1536 -> 4 slices of 384).
Core c = b*4 + j handles batch b, channels [j*384,(j+1)*384), full L=2048.
Cross-core comms: 4 token-chunked AllReduces of the x_proj partial
([80,512] each) and ReduceScatter of out_proj partials (each core ends
up with a 512-token slice of the final hidden states).

The whole layer is pipelined over 512-token blocks: for each block nb,
in_proj -> conv -> x_proj -> AllReduce -> delta -> selective scan ->
gate/finalize -> out_proj columns all run inside one loop iteration, so
the PE/Act/Pool/DMA work of block nb+1 overlaps the DVE-bound scan of
block nb. Engine assignment (DVE is the bottleneck):
  LN stats on Act (Copy/Square + accum_out), tiny LN scalars on DVE,
  LN apply on Act, xn transposed via DMA xbar transpose; in_proj /
  x_proj / dt / out_proj matmuls in bf16 on PE; conv = 4 DVE bf16
  tensor_scalar muls + 3 Pool adds + Act SiLU; B/C rows replicated
  across partitions by DMA broadcast from DRAM; dA = Exp(A_n*delta) on
  Act; dBx = (delta*xc) x B on Pool (bf16 TensorTensor); the 16
  tensor_tensor_scans per (block, d-tile) on DVE (fp32 carried state);
  y = sum_n h*C via DVE bf16 mul + tree-adds; +D_skip*xc, *silu(z) on
  DVE; ReduceScatter + per-m output DMA tail.
"""

import os
import sys

sys.path.insert(0, "/opt/trn_rl_repo")

import numpy as np
import ml_dtypes
from contextlib import ExitStack

import concourse.bass as bass
import concourse.bacc as bacc
import concourse.mybir as mybir
import concourse.tile as tile
from concourse.bass_utils import run_bass_kernel_spmd

F = mybir.dt.float32
BF = mybir.dt.bfloat16
FR = mybir.dt.float32r
AF = mybir.ActivationFunctionType
OP = mybir.AluOpType

B, L, DM = 2, 2048, 768
DI, DS, DC, DTR = 1536, 16, 4, 48
SL = 384          # channel slice per core
NJ = 3            # d-tiles of 128 per core
KT = DM // 128    # 6 contraction tiles for in_proj
TB = 512          # token block
NBLK = L // TB
NH = DS // 2      # states per half
NT = L // 128     # LN token tiles
NCORES = 8
GROUPS = [[0, 1, 2, 3], [4, 5, 6, 7]]
LN_EPS = 1e-5
TOK = L // 4      # token slice per core for outputs

_CACHE = {}


def _build(single=False):
    key = "nc1" if single else "nc"
    if key in _CACHE:
        return _CACHE[key]

    nc = bacc.Bacc("TRN2", target_bir_lowering=False, debug=False,
                   num_devices=1 if single else NCORES)

    # ---------------- I/O ----------------
    x_b = nc.dram_tensor("x_b", [L, DM], F, kind="ExternalInput").ap()
    res_x = nc.dram_tensor("res_x", [TOK, DM], F, kind="ExternalInput").ap()
    res_in = nc.dram_tensor("res_in", [TOK, DM], F, kind="ExternalInput").ap()
    W_inT = nc.dram_tensor("W_inT", [DM, 2 * SL], BF, kind="ExternalInput").ap()
    bias_in = nc.dram_tensor("bias_in", [2 * SL], F, kind="ExternalInput").ap()
    WxT = nc.dram_tensor("WxT", [SL, 80], BF, kind="ExternalInput").ap()
    WdtT = nc.dram_tensor("WdtT", [DTR, SL], F, kind="ExternalInput").ap()
    bdt = nc.dram_tensor("bdt", [SL], F, kind="ExternalInput").ap()
    Acols = nc.dram_tensor("Acols", [SL, DS], F, kind="ExternalInput").ap()
    convw = nc.dram_tensor("convw", [SL, DC], F, kind="ExternalInput").ap()
    convb = nc.dram_tensor("convb", [SL], F, kind="ExternalInput").ap()
    Dskip = nc.dram_tensor("Dskip", [SL], F, kind="ExternalInput").ap()
    WoT = nc.dram_tensor("WoT", [SL, DM], BF, kind="ExternalInput").ap()
    hid_out = nc.dram_tensor("hid_out", [DM, TOK], F, kind="ExternalOutput").ap()
    res_out = nc.dram_tensor("res_out", [TOK, DM], F, kind="ExternalOutput").ap()

    with tile.TileContext(nc, trace_sim=False) as tc, ExitStack() as top:
        dram = top.enter_context(tc.tile_pool(name="dram", bufs=1, space="DRAM"))
        proj_part = dram.tile([4 * 80, TB], F)
        proj_sum = dram.tile([4 * 80, TB], F)
        bc_bf = dram.tile([2 * DS, L], BF)  # B/C rows in bf16
        op_part = dram.tile([4 * DM, TOK], F)
        op_rs = dram.tile([DM, TOK], F)

        const = top.enter_context(tc.tile_pool(name="const", bufs=1))
        bias_sb = const.tile([128, 6], F)     # col m: bias_in[m*128+p]
        nc.sync.dma_start(bias_sb[:], bias_in.rearrange("(m p) -> p m", p=128))
        acol_sb = const.tile([128, NJ * DS], F)  # col j*16+n: A[j*128+p, n]
        nc.sync.dma_start(acol_sb[:].rearrange("p (j n) -> p j n", j=NJ),
                          Acols.rearrange("(j p) n -> p j n", p=128))
        convw_sb = const.tile([128, NJ * DC], F)
        nc.sync.dma_start(convw_sb[:].rearrange("p (j k) -> p j k", j=NJ),
                          convw.rearrange("(j p) k -> p j k", p=128))
        convb_sb = const.tile([128, NJ], F)
        nc.sync.dma_start(convb_sb[:], convb.rearrange("(j p) -> p j", p=128))
        dskip_sb = const.tile([128, NJ], F)
        nc.sync.dma_start(dskip_sb[:], Dskip.rearrange("(j p) -> p j", p=128))
        bdt_sb = const.tile([128, NJ], F)
        nc.sync.dma_start(bdt_sb[:], bdt.rearrange("(j p) -> p j", p=128))

        persist = top.enter_context(tc.tile_pool(name="persist", bufs=1))
        carry = [persist.tile([128, DS], F, tag=f"cr{j}", name=f"cr{j}")
                 for j in range(NJ)]
        for j in range(NJ):
            nc.gpsimd.memset(carry[j][:], 0.0)
        x_pad = [persist.tile([128, L + DC - 1], BF, tag=f"xp{j}", name=f"xp{j}")
                 for j in range(NJ)]
        for j in range(NJ):
            nc.gpsimd.memset(x_pad[j][:, 0:DC - 1], 0.0)
        xnT_all = persist.tile([128, KT * L], BF)
        winT_sb = [persist.tile([128, 2 * SL], BF, tag=f"wi{k}", name=f"wi{k}")
                   for k in range(KT)]
        for k in range(KT):
            nc.sync.dma_start(winT_sb[k][:], W_inT[bass.ts(k, 128), :])
        wxT_sb = [persist.tile([128, 80], BF, tag=f"wx{j}", name=f"wx{j}")
                  for j in range(NJ)]
        for j in range(NJ):
            nc.sync.dma_start(wxT_sb[j][:], WxT[bass.ts(j, 128), :])
        wdtT_sb = persist.tile([DTR, SL], FR, tag="wdt")
        nc.sync.dma_start(wdtT_sb[:], WdtT.bitcast(FR))
        woT_sb = [persist.tile([128, DM], BF, tag=f"wo{j}", name=f"wo{j}")
                  for j in range(NJ)]
        for j in range(NJ):
            nc.sync.dma_start(woT_sb[j][:], WoT[bass.ts(j, 128), :])

        # ============ LayerNorm + transpose (two-pass pipeline) ============
        with tc.tile_pool(name="lnp", bufs=1) as lpp, \
             tc.tile_pool(name="ln", bufs=3) as lp:
            xts, mvs, vss = [], [], []
            for tt in range(NT):
                xt = lpp.tile([128, DM], F, tag=f"xt{tt}", name=f"xt{tt}")
                nc.sync.dma_start(xt[:], x_b[bass.ts(tt, 128), :])
                st6 = lp.tile([128, 2 * 6], F, tag="st6")
                nc.vector.bn_stats(st6[:, 0:6], xt[:, 0:DM // 2])
                nc.vector.bn_stats(st6[:, 6:12], xt[:, DM // 2:DM])
                mv = lpp.tile([128, 2], F, tag=f"mv{tt}", name=f"mv{tt}")
                nc.vector.bn_aggr(mv[:], st6[:])
                vs = lpp.tile([128, 1], F, tag=f"vs{tt}", name=f"vs{tt}")
                nc.vector.tensor_scalar(out=vs[:], in0=mv[:, 1:2],
                                        scalar1=1.0, scalar2=LN_EPS,
                                        op0=OP.mult, op1=OP.add)
                xts.append(xt); mvs.append(mv); vss.append(vs)
            sds = []
            for tt in range(NT):
                sd = lpp.tile([128, 1], F, tag=f"sd{tt}", name=f"sd{tt}")
                nc.scalar.activation(sd[:], vss[tt][:], AF.Sqrt)
                sds.append(sd)
            for tt in range(NT):
                rstd = lp.tile([128, 1], F, tag="rstd")
                nc.vector.reciprocal(rstd[:], sds[tt][:])
                nmr = lp.tile([128, 1], F, tag="nmr")
                nc.vector.tensor_scalar(out=nmr[:], in0=mvs[tt][:, 0:1],
                                        scalar1=rstd[:, 0:1], scalar2=-1.0,
                                        op0=OP.mult, op1=OP.mult)
                xn = lp.tile([128, DM], BF, tag="xn")
                nc.vector.tensor_scalar(out=xn[:], in0=xts[tt][:],
                                        scalar1=rstd[:, 0:1],
                                        scalar2=nmr[:, 0:1],
                                        op0=OP.mult, op1=OP.add)
                nc.sync.dma_start_transpose(
                    xnT_all[:].rearrange("p (k t) -> p k t", k=KT)
                    [:, :, bass.ts(tt, 128)], xn[:])

        # ---- residual output (independent; fills early DMA/Pool idle) ----
        with tc.tile_pool(name="res", bufs=2) as rp:
            for t4 in range(TOK // 128):
                rx = rp.tile([128, DM], F)
                rr = rp.tile([128, DM], F)
                ro = rp.tile([128, DM], F)
                nc.sync.dma_start(rx[:], res_x[bass.ts(t4, 128), :])
                nc.sync.dma_start(rr[:], res_in[bass.ts(t4, 128), :])
                nc.gpsimd.tensor_tensor(out=ro[:], in0=rx[:], in1=rr[:],
                                        op=OP.add)
                nc.sync.dma_start(res_out[bass.ts(t4, 128), :], ro[:])

        # ============ main per-block pipeline ============
        mb = ExitStack()
        with mb:
            bkp = mb.enter_context(tc.tile_pool(name="blk", bufs=2))
            dtp = mb.enter_context(tc.tile_pool(name="dtb", bufs=1))
            nbp = mb.enter_context(tc.tile_pool(name="nbtmp", bufs=1))
            cp = mb.enter_context(tc.tile_pool(name="conv", bufs=1))
            ipp = mb.enter_context(tc.tile_pool(name="ippsum", bufs=2, space="PSUM"))
            xps = mb.enter_context(tc.tile_pool(name="xpps", bufs=1, space="PSUM"))
            dps = mb.enter_context(tc.tile_pool(name="dtps", bufs=2, space="PSUM"))
            ops = mb.enter_context(tc.tile_pool(name="opps", bufs=2, space="PSUM"))
            bp = mb.enter_context(tc.tile_pool(name="brep", bufs=2))
            adp = mb.enter_context(tc.tile_pool(name="sdA", bufs=1))
            dbp = mb.enter_context(tc.tile_pool(name="sdbx", bufs=2))
            ypp = mb.enter_context(tc.tile_pool(name="syp", bufs=1))
            hp = mb.enter_context(tc.tile_pool(name="sh", bufs=2))
            up = mb.enter_context(tc.tile_pool(name="su", bufs=2))
            ytp = mb.enter_context(tc.tile_pool(name="syt", bufs=1))
            fp = mb.enter_context(tc.tile_pool(name="fin", bufs=1))
            op_ = mb.enter_context(tc.tile_pool(name="oproj", bufs=2))
            xc_t, z_t, dl_t, y_t, ub_t = {}, {}, {}, {}, {}

            def emit_outproj(nb):
                for m in range(6):
                    ps = ops.tile([128, TB], F, name="ps_op")
                    for j in range(NJ):
                        nc.tensor.matmul(ps[:], woT_sb[j][:, bass.ts(m, 128)],
                                         y_t[(nb, j)][:],
                                         start=(j == 0), stop=(j == NJ - 1))
                    otn = op_.tile([128, TB], F, tag="otn", name="otn")
                    nc.scalar.copy(otn[:], ps[:])
                    nc.sync.dma_start(
                        op_part[(m * 4 + nb) * 128:(m * 4 + nb + 1) * 128, :],
                        otn[:])

            def emit_front(nb):
                """in_proj -> conv -> x_proj -> AllReduce -> delta/ub for block nb."""
                tsl = slice(nb * TB, (nb + 1) * TB)
                for m in range(6):
                    ps = ipp.tile([128, TB], F, name="ps_ip")
                    for k in range(KT):
                        nc.tensor.matmul(ps[:],
                                         winT_sb[k][:, bass.ts(m, 128)],
                                         xnT_all[:, k * L + nb * TB:
                                                 k * L + (nb + 1) * TB],
                                         start=(k == 0), stop=(k == KT - 1))
                    if m < 3:
                        nc.scalar.activation(
                            x_pad[m][:, DC - 1 + nb * TB:
                                     DC - 1 + (nb + 1) * TB],
                            ps[:], AF.Identity, bias=bias_sb[:, m:m + 1])
                    else:
                        z = bkp.tile([128, TB], BF, tag=f"z{m - 3}",
                                     name=f"z{m - 3}")
                        nc.scalar.activation(z[:], ps[:], AF.Silu,
                                             bias=bias_sb[:, m:m + 1])
                        z_t[(nb, m - 3)] = z
                for j in range(NJ):
                    terms = []
                    for k in range(DC):
                        ak = cp.tile([128, TB], BF, tag=f"cm{k}", name=f"cm{k}")
                        nc.vector.tensor_scalar(
                            out=ak[:], in0=x_pad[j][:, nb * TB + k:
                                                    nb * TB + k + TB],
                            scalar1=convw_sb[:, j * DC + k:j * DC + k + 1],
                            scalar2=None, op0=OP.mult)
                        terms.append(ak)
                    s01 = cp.tile([128, TB], BF, tag="s01")
                    nc.gpsimd.tensor_tensor(out=s01[:], in0=terms[0][:],
                                            in1=terms[1][:], op=OP.add)
                    s23 = cp.tile([128, TB], BF, tag="s23")
                    nc.gpsimd.tensor_tensor(out=s23[:], in0=terms[2][:],
                                            in1=terms[3][:], op=OP.add)
                    st = cp.tile([128, TB], BF, tag="st")
                    nc.gpsimd.tensor_tensor(out=st[:], in0=s01[:],
                                            in1=s23[:], op=OP.add)
                    xc = bkp.tile([128, TB], BF, tag=f"xc{j}", name=f"xc{j}")
                    nc.scalar.activation(xc[:], st[:], AF.Silu,
                                         bias=convb_sb[:, j:j + 1])
                    xc_t[(nb, j)] = xc
                ps = xps.tile([80, TB], F, name="ps_xp")
                for j in range(NJ):
                    nc.tensor.matmul(ps[:], wxT_sb[j][:], xc_t[(nb, j)][:],
                                     start=(j == 0), stop=(j == NJ - 1))
                pp = nbp.tile([80, TB], F, tag="pp", name="pp")
                nc.scalar.copy(pp[:], ps[:])
                nc.sync.dma_start(proj_part[bass.ts(nb, 80), :], pp[:])
                if single:
                    nc.sync.dma_start(proj_sum[bass.ts(nb, 80), :],
                                      proj_part[bass.ts(nb, 80), :])
                else:
                    nc.gpsimd.collective_compute(
                        "AllReduce", OP.add, replica_groups=GROUPS,
                        ins=[proj_part[nb * 80:(nb + 1) * 80, :].opt()],
                        outs=[proj_sum[nb * 80:(nb + 1) * 80, :].opt()])
                bc32 = nbp.tile([2 * DS, TB], F, tag="bc32", name="bc32")
                nc.sync.dma_start(bc32[:],
                                  proj_sum[nb * 80 + DTR:
                                           nb * 80 + DTR + 2 * DS, :])
                bcb = nbp.tile([2 * DS, TB], BF, tag="bcb", name="bcb")
                nc.scalar.copy(bcb[:], bc32[:])
                nc.sync.dma_start(bc_bf[:, tsl], bcb[:])
                dtT = dtp.tile([DTR, TB], FR, tag="dtT", name="dtT")
                nc.sync.dma_start(dtT[:],
                                  proj_sum[nb * 80:nb * 80 + DTR, :]
                                  .bitcast(FR))
                ets = []
                for j in range(NJ):
                    psd = dps.tile([128, TB], F, name="psd")
                    nc.tensor.matmul(psd[:], wdtT_sb[:, bass.ts(j, 128)],
                                     dtT[:], start=True, stop=True)
                    et = nbp.tile([128, TB], F, tag=f"et{j}", name=f"et{j}")
                    nc.scalar.activation(et[:], psd[:], AF.Exp,
                                         bias=bdt_sb[:, j:j + 1])
                    ets.append(et)
                for j in range(NJ):
                    dl = bkp.tile([128, TB], BF, tag=f"dl{j}", name=f"dl{j}")
                    nc.scalar.activation(dl[:], ets[j][:], AF.Ln, bias=1.0)
                    dl_t[(nb, j)] = dl
                    ub = up.tile([128, TB], BF, tag=f"ub{j}", name=f"ub{j}")
                    nc.vector.tensor_mul(ub[:], dl[:], xc_t[(nb, j)][:])
                    ub_t[(nb, j)] = ub

            def emit_scan_half(nb, half):
                tsl = slice(nb * TB, (nb + 1) * TB)
                bc_rep = bp.tile([128, 2 * NH * TB], BF, tag="bcr", name="bcr")
                nc.sync.dma_start(
                    bc_rep[:, 0:NH * TB].rearrange("p (n t) -> p n t", n=NH),
                    bc_bf[half * NH:(half + 1) * NH, tsl]
                    .unsqueeze(0).broadcast_to([128, NH, TB]))
                nc.sync.dma_start(
                    bc_rep[:, NH * TB:2 * NH * TB]
                    .rearrange("p (n t) -> p n t", n=NH),
                    bc_bf[DS + half * NH:DS + (half + 1) * NH, tsl]
                    .unsqueeze(0).broadcast_to([128, NH, TB]))
                b_rep = bc_rep[:, 0:NH * TB]
                c_rep = bc_rep[:, NH * TB:2 * NH * TB]
                for j in range(NJ):
                    dA = adp.tile([128, NH * TB], F, tag="dA", name="dA")
                    for n in range(NH):
                        nc.scalar.activation(
                            dA[:, bass.ts(n, TB)], dl_t[(nb, j)][:], AF.Exp,
                            scale=acol_sb[:, j * DS + half * NH + n:
                                          j * DS + half * NH + n + 1])
                    dbx = dbp.tile([128, NH * TB], BF, tag="dbx", name="dbx")
                    for hv in range(2):
                        sl4 = slice(hv * NH // 2 * TB,
                                    (hv + 1) * NH // 2 * TB)
                        nc.gpsimd.tensor_tensor(
                            out=dbx[:, sl4].rearrange(
                                "p (n t) -> p n t", n=NH // 2),
                            in0=ub_t[(nb, j)][:].unsqueeze(1)
                            .broadcast_to([128, NH // 2, TB]),
                            in1=b_rep[:, sl4].rearrange(
                                "p (n t) -> p n t", n=NH // 2),
                            op=OP.mult)
                    h = hp.tile([128, NH * TB], BF, tag="h", name="h")
                    for n in range(NH):
                        nc.vector.tensor_tensor_scan(
                            out=h[:, bass.ts(n, TB)],
                            data0=dA[:, bass.ts(n, TB)],
                            data1=dbx[:, bass.ts(n, TB)],
                            initial=carry[j][:, half * NH + n:
                                             half * NH + n + 1],
                            op0=OP.mult, op1=OP.add)
                    nc.vector.tensor_copy(
                        carry[j][:, half * NH:(half + 1) * NH],
                        h[:].rearrange("p (n t) -> p n t",
                                       n=NH)[:, :, TB - 1])
                    yp = ypp.tile([128, NH * TB], BF, tag="yp", name="yp")
                    nc.vector.tensor_mul(yp[:], h[:], c_rep)
                    t2 = ytp.tile([128, 4 * TB], BF, tag="t2", name="t2")
                    nc.vector.tensor_add(t2[:], yp[:, 0:4 * TB],
                                         yp[:, 4 * TB:8 * TB])
                    t3 = ytp.tile([128, 2 * TB], BF, tag="t3", name="t3")
                    nc.vector.tensor_add(t3[:], t2[:, 0:2 * TB],
                                         t2[:, 2 * TB:4 * TB])
                    if half == 0:
                        y = bkp.tile([128, TB], BF, tag=f"y{j}", name=f"y{j}")
                        nc.vector.tensor_add(y[:], t3[:, 0:TB],
                                             t3[:, TB:2 * TB])
                        y_t[(nb, j)] = y
                    else:
                        yt = ytp.tile([128, TB], BF, tag="yt", name="yt")
                        nc.vector.tensor_add(yt[:], t3[:, 0:TB],
                                             t3[:, TB:2 * TB])
                        nc.vector.tensor_add(y_t[(nb, j)][:],
                                             y_t[(nb, j)][:], yt[:])

            def emit_fin(nb):
                for j in range(NJ):
                    t0 = fp.tile([128, TB], BF, tag="t0", name="t0")
                    nc.vector.tensor_scalar(out=t0[:], in0=xc_t[(nb, j)][:],
                                            scalar1=dskip_sb[:, j:j + 1],
                                            scalar2=None, op0=OP.mult)
                    t1 = fp.tile([128, TB], BF, tag="t1", name="t1")
                    nc.vector.tensor_add(t1[:], t0[:], y_t[(nb, j)][:])
                    nc.vector.tensor_mul(y_t[(nb, j)][:], t1[:],
                                         z_t[(nb, j)][:])

            emit_front(0)
            for nb in range(4):
                emit_scan_half(nb, 0)
                if nb + 1 < 4:
                    emit_front(nb + 1)
                if nb > 0:
                    emit_outproj(nb - 1)
                emit_scan_half(nb, 1)
                emit_fin(nb)

            emit_outproj(3)

            # ---- ReduceScatter + output tail ----
            for m in range(6):
                if single:
                    nc.sync.dma_start(op_rs[bass.ts(m, 128), :],
                                      op_part[m * 512:m * 512 + 128, :])
                else:
                    nc.gpsimd.collective_compute(
                        "ReduceScatter", OP.add, replica_groups=GROUPS,
                        ins=[op_part[m * 512:(m + 1) * 512, :].opt()],
                        outs=[op_rs[bass.ts(m, 128), :].opt()])
                nc.sync.dma_start(hid_out[bass.ts(m, 128), :],
                                  op_rs[bass.ts(m, 128), :])

    nc.compile()
    _CACHE[key] = nc
    return nc


def _prep_inputs(inp):
    gamma, beta = inp["ln_gamma"], inp["ln_beta"]
    W_in = inp["W_in"]
    W_in_f = W_in * gamma[None, :]
    bias_full = W_in @ beta            # [2*DI]
    A = -np.exp(inp["A_log"])          # [DI, DS]
    bf = ml_dtypes.bfloat16

    in_maps = []
    for c in range(NCORES):
        b, j = c // 4, c % 4
        S = slice(j * SL, (j + 1) * SL)
        rows = np.r_[j * SL:(j + 1) * SL, DI + j * SL:DI + (j + 1) * SL]
        m = {
            "x_b": inp["x"][b],
            "res_x": inp["x"][b, j * TOK:(j + 1) * TOK],
            "res_in": inp["residual"][b, j * TOK:(j + 1) * TOK],
            "W_inT": np.ascontiguousarray(W_in_f[rows].T).astype(bf),
            "bias_in": np.ascontiguousarray(bias_full[rows]),
            "WxT": np.ascontiguousarray(inp["W_xproj"][:, S].T).astype(bf),
            "WdtT": np.ascontiguousarray(inp["W_dt"][S].T),
            "bdt": np.ascontiguousarray(inp["b_dt"][S]),
            "Acols": np.ascontiguousarray(A[S]),
            "convw": np.ascontiguousarray(inp["conv_w"][S]),
            "convb": np.ascontiguousarray(inp["conv_b"][S]),
            "Dskip": np.ascontiguousarray(inp["D_skip"][S]),
            "WoT": np.ascontiguousarray(inp["W_out"][:, S].T).astype(bf),
        }
        in_maps.append(m)
    return in_maps


def _assemble(results):
    hidden = np.empty((B, L, DM), np.float32)
    residual = np.empty((B, L, DM), np.float32)
    for c in range(NCORES):
        b, j = c // 4, c % 4
        r = results[c]
        hidden[b, j * TOK:(j + 1) * TOK] = r["hid_out"].T
        residual[b, j * TOK:(j + 1) * TOK] = r["res_out"]
    return hidden, residual


def kernel(**inputs):
    inp = {k: np.ascontiguousarray(np.asarray(v, dtype=np.float32))
           for k, v in inputs.items()}
    nc = _build()
    in_maps = _prep_inputs(inp)
    res = run_bass_kernel_spmd(nc, in_maps, list(range(NCORES)))
    return _assemble(res.results)


# revision 22
# speedup vs baseline: 1.1052x; 1.0050x over previous
"""Mamba BasicBlock kernel for 8 Trainium2 NeuronCores.

Sharding: 2 batches x 4 channel-slices (D_INNER 1536 -> 4 slices of 384).
Core c = b*4 + j handles batch b, channels [j*384,(j+1)*384), full L=2048.
Cross-core comms: 4 token-chunked AllReduces of the x_proj partial
([80,512] each) and ReduceScatter of out_proj partials (each core ends
up with a 512-token slice of the final hidden states).

The whole layer is pipelined over 512-token blocks: for each block nb,
in_proj -> conv -> x_proj -> AllReduce -> delta -> selective scan ->
gate/finalize -> out_proj columns all run inside one loop iteration, so
the PE/Act/Pool/DMA work of block nb+1 overlaps the DVE-bound scan of
block nb. Engine assignment (DVE is the bottleneck):
  LN stats on Act (Copy/Square + accum_out), tiny LN scalars on DVE,
  LN apply on Act, xn transposed via DMA xbar transpose; in_proj /
  x_proj / dt / out_proj matmuls in bf16 on PE; conv = 4 DVE bf16
  tensor_scalar muls + 3 Pool adds + Act SiLU; B/C rows replicated
  across partitions by DMA broadcast from DRAM; dA = Exp(A_n*delta) on
  Act; dBx = (delta*xc) x B on Pool (bf16 TensorTensor); the 16
  tensor_tensor_scans per (block, d-tile) on DVE (fp32 carried state);
  y = sum_n h*C via DVE bf16 mul + tree-adds; +D_skip*xc, *silu(z) on
  DVE; ReduceScatter + per-m output DMA tail.
"""

import os
import sys

sys.path.insert(0, "/opt/trn_rl_repo")

import numpy as np
import ml_dtypes
from contextlib import ExitStack

import concourse.bass as bass
import concourse.bacc as bacc
import concourse.mybir as mybir
import concourse.tile as tile
from concourse.bass_utils import run_bass_kernel_spmd

F = mybir.dt.float32
BF = mybir.dt.bfloat16
FR = mybir.dt.float32r
AF = mybir.ActivationFunctionType
OP = mybir.AluOpType

B, L, DM = 2, 2048, 768
DI, DS, DC, DTR = 1536, 16, 4, 48
SL = 384          # channel slice per core
NJ = 3            # d-tiles of 128 per core
KT = DM // 128    # 6 contraction tiles for in_proj
TB = 512          # token block
NBLK = L // TB
NH = DS // 2      # states per half
NT = L // 128     # LN token tiles
NCORES = 8
GROUPS = [[0, 1, 2, 3], [4, 5, 6, 7]]
LN_EPS = 1e-5
TOK = L // 4      # token slice per core for outputs

_CACHE = {}


def _build(single=False):
    key = "nc1" if single else "nc"
    if key in _CACHE:
        return _CACHE[key]

    nc = bacc.Bacc("TRN2", target_bir_lowering=False, debug=False,
                   num_devices=1 if single else NCORES)

    # ---------------- I/O ----------------
    x_b = nc.dram_tensor("x_b", [L, DM], F, kind="ExternalInput").ap()
    res_x = nc.dram_tensor("res_x", [TOK, DM], F, kind="ExternalInput").ap()
    res_in = nc.dram_tensor("res_in", [TOK, DM], F, kind="ExternalInput").ap()
    W_inT = nc.dram_tensor("W_inT", [DM, 2 * SL], BF, kind="ExternalInput").ap()
    bias_in = nc.dram_tensor("bias_in", [2 * SL], F, kind="ExternalInput").ap()
    WxT = nc.dram_tensor("WxT", [SL, 80], BF, kind="ExternalInput").ap()
    WdtT = nc.dram_tensor("WdtT", [DTR, SL], F, kind="ExternalInput").ap()
    bdt = nc.dram_tensor("bdt", [SL], F, kind="ExternalInput").ap()
    Acols = nc.dram_tensor("Acols", [SL, DS], F, kind="ExternalInput").ap()
    convw = nc.dram_tensor("convw", [SL, DC], F, kind="ExternalInput").ap()
    convb = nc.dram_tensor("convb", [SL], F, kind="ExternalInput").ap()
    Dskip = nc.dram_tensor("Dskip", [SL], F, kind="ExternalInput").ap()
    WoT = nc.dram_tensor("WoT", [SL, DM], BF, kind="ExternalInput").ap()
    hid_out = nc.dram_tensor("hid_out", [DM, TOK], F, kind="ExternalOutput").ap()
    res_out = nc.dram_tensor("res_out", [TOK, DM], F, kind="ExternalOutput").ap()

    with tile.TileContext(nc, trace_sim=False) as tc, ExitStack() as top:
        dram = top.enter_context(tc.tile_pool(name="dram", bufs=1, space="DRAM"))
        proj_part = dram.tile([4 * 80, TB], F)
        proj_sum = dram.tile([4 * 80, TB], F)
        bc_bf = dram.tile([2 * DS, L], BF)  # B/C rows in bf16
        op_part = dram.tile([4 * DM, TOK], F)
        op_rs = dram.tile([DM, TOK], F)

        const = top.enter_context(tc.tile_pool(name="const", bufs=1))
        bias_sb = const.tile([128, 6], F)     # col m: bias_in[m*128+p]
        nc.sync.dma_start(bias_sb[:], bias_in.rearrange("(m p) -> p m", p=128))
        acol_sb = const.tile([128, NJ * DS], F)  # col j*16+n: A[j*128+p, n]
        nc.sync.dma_start(acol_sb[:].rearrange("p (j n) -> p j n", j=NJ),
                          Acols.rearrange("(j p) n -> p j n", p=128))
        convw_sb = const.tile([128, NJ * DC], F)
        nc.sync.dma_start(convw_sb[:].rearrange("p (j k) -> p j k", j=NJ),
                          convw.rearrange("(j p) k -> p j k", p=128))
        convb_sb = const.tile([128, NJ], F)
        nc.sync.dma_start(convb_sb[:], convb.rearrange("(j p) -> p j", p=128))
        dskip_sb = const.tile([128, NJ], F)
        nc.sync.dma_start(dskip_sb[:], Dskip.rearrange("(j p) -> p j", p=128))
        bdt_sb = const.tile([128, NJ], F)
        nc.sync.dma_start(bdt_sb[:], bdt.rearrange("(j p) -> p j", p=128))

        persist = top.enter_context(tc.tile_pool(name="persist", bufs=1))
        carry = [persist.tile([128, DS], F, tag=f"cr{j}", name=f"cr{j}")
                 for j in range(NJ)]
        for j in range(NJ):
            nc.gpsimd.memset(carry[j][:], 0.0)
        x_pad = [persist.tile([128, L + DC - 1], BF, tag=f"xp{j}", name=f"xp{j}")
                 for j in range(NJ)]
        for j in range(NJ):
            nc.gpsimd.memset(x_pad[j][:, 0:DC - 1], 0.0)
        xnT_all = persist.tile([128, KT * L], BF)
        winT_sb = [persist.tile([128, 2 * SL], BF, tag=f"wi{k}", name=f"wi{k}")
                   for k in range(KT)]
        for k in range(KT):
            nc.sync.dma_start(winT_sb[k][:], W_inT[bass.ts(k, 128), :])
        wxT_sb = [persist.tile([128, 80], BF, tag=f"wx{j}", name=f"wx{j}")
                  for j in range(NJ)]
        for j in range(NJ):
            nc.sync.dma_start(wxT_sb[j][:], WxT[bass.ts(j, 128), :])
        wdtT_sb = persist.tile([DTR, SL], FR, tag="wdt")
        nc.sync.dma_start(wdtT_sb[:], WdtT.bitcast(FR))
        woT_sb = [persist.tile([128, DM], BF, tag=f"wo{j}", name=f"wo{j}")
                  for j in range(NJ)]
        for j in range(NJ):
            nc.sync.dma_start(woT_sb[j][:], WoT[bass.ts(j, 128), :])

        # ============ LayerNorm + transpose (two-pass pipeline) ============
        with tc.tile_pool(name="lnp", bufs=1) as lpp, \
             tc.tile_pool(name="ln", bufs=3) as lp:
            xts, mvs, vss = [], [], []
            for tt in range(NT):
                xt = lpp.tile([128, DM], F, tag=f"xt{tt}", name=f"xt{tt}")
                nc.sync.dma_start(xt[:], x_b[bass.ts(tt, 128), :])
                st6 = lp.tile([128, 2 * 6], F, tag="st6")
                nc.vector.bn_stats(st6[:, 0:6], xt[:, 0:DM // 2])
                nc.vector.bn_stats(st6[:, 6:12], xt[:, DM // 2:DM])
                mv = lpp.tile([128, 2], F, tag=f"mv{tt}", name=f"mv{tt}")
                nc.vector.bn_aggr(mv[:], st6[:])
                vs = lpp.tile([128, 1], F, tag=f"vs{tt}", name=f"vs{tt}")
                nc.vector.tensor_scalar(out=vs[:], in0=mv[:, 1:2],
                                        scalar1=1.0, scalar2=LN_EPS,
                                        op0=OP.mult, op1=OP.add)
                xts.append(xt); mvs.append(mv); vss.append(vs)
            sds = []
            for tt in range(NT):
                sd = lpp.tile([128, 1], F, tag=f"sd{tt}", name=f"sd{tt}")
                nc.scalar.activation(sd[:], vss[tt][:], AF.Sqrt)
                sds.append(sd)
            for tt in range(NT):
                rstd = lp.tile([128, 1], F, tag="rstd")
                nc.vector.reciprocal(rstd[:], sds[tt][:])
                nmr = lp.tile([128, 1], F, tag="nmr")
                nc.vector.tensor_scalar(out=nmr[:], in0=mvs[tt][:, 0:1],
                                        scalar1=rstd[:, 0:1], scalar2=-1.0,
                                        op0=OP.mult, op1=OP.mult)
                xn = lp.tile([128, DM], BF, tag="xn")
                nc.vector.tensor_scalar(out=xn[:], in0=xts[tt][:],
                                        scalar1=rstd[:, 0:1],
                                        scalar2=nmr[:, 0:1],
                                        op0=OP.mult, op1=OP.add)
                nc.sync.dma_start_transpose(
                    xnT_all[:].rearrange("p (k t) -> p k t", k=KT)
                    [:, :, bass.ts(tt, 128)], xn[:])

        # ---- residual output (independent; fills early DMA/Pool idle) ----
        with tc.tile_pool(name="res", bufs=2) as rp:
            for t4 in range(TOK // 128):
                rx = rp.tile([128, DM], F)
                rr = rp.tile([128, DM], F)
                ro = rp.tile([128, DM], F)
                nc.sync.dma_start(rx[:], res_x[bass.ts(t4, 128), :])
                nc.sync.dma_start(rr[:], res_in[bass.ts(t4, 128), :])
                nc.gpsimd.tensor_tensor(out=ro[:], in0=rx[:], in1=rr[:],
                                        op=OP.add)
                nc.sync.dma_start(res_out[bass.ts(t4, 128), :], ro[:])

        # ============ main per-block pipeline ============
        mb = ExitStack()
        with mb:
            bkp = mb.enter_context(tc.tile_pool(name="blk", bufs=2))
            dtp = mb.enter_context(tc.tile_pool(name="dtb", bufs=1))
            nbp = mb.enter_context(tc.tile_pool(name="nbtmp", bufs=1))
            cp = mb.enter_context(tc.tile_pool(name="conv", bufs=1))
            ipp = mb.enter_context(tc.tile_pool(name="ippsum", bufs=2, space="PSUM"))
            xps = mb.enter_context(tc.tile_pool(name="xpps", bufs=1, space="PSUM"))
            dps = mb.enter_context(tc.tile_pool(name="dtps", bufs=2, space="PSUM"))
            ops = mb.enter_context(tc.tile_pool(name="opps", bufs=2, space="PSUM"))
            bp = mb.enter_context(tc.tile_pool(name="brep", bufs=2))
            adp = mb.enter_context(tc.tile_pool(name="sdA", bufs=2))
            dbp = mb.enter_context(tc.tile_pool(name="sdbx", bufs=2))
            ypp = mb.enter_context(tc.tile_pool(name="syp", bufs=1))
            hp = mb.enter_context(tc.tile_pool(name="sh", bufs=2))
            up = mb.enter_context(tc.tile_pool(name="su", bufs=2))
            ytp = mb.enter_context(tc.tile_pool(name="syt", bufs=1))
            fp = mb.enter_context(tc.tile_pool(name="fin", bufs=1))
            op_ = mb.enter_context(tc.tile_pool(name="oproj", bufs=2))
            xc_t, z_t, dl_t, y_t, ub_t = {}, {}, {}, {}, {}

            def emit_outproj(nb):
                for m in range(6):
                    ps = ops.tile([128, TB], F, name="ps_op")
                    for j in range(NJ):
                        nc.tensor.matmul(ps[:], woT_sb[j][:, bass.ts(m, 128)],
                                         y_t[(nb, j)][:],
                                         start=(j == 0), stop=(j == NJ - 1))
                    otn = op_.tile([128, TB], F, tag="otn", name="otn")
                    nc.scalar.copy(otn[:], ps[:])
                    nc.sync.dma_start(
                        op_part[(m * 4 + nb) * 128:(m * 4 + nb + 1) * 128, :],
                        otn[:])

            def emit_front(nb):
                """in_proj -> conv -> x_proj -> AllReduce -> delta/ub for block nb."""
                tsl = slice(nb * TB, (nb + 1) * TB)
                for m in range(6):
                    ps = ipp.tile([128, TB], F, name="ps_ip")
                    for k in range(KT):
                        nc.tensor.matmul(ps[:],
                                         winT_sb[k][:, bass.ts(m, 128)],
                                         xnT_all[:, k * L + nb * TB:
                                                 k * L + (nb + 1) * TB],
                                         start=(k == 0), stop=(k == KT - 1))
                    if m < 3:
                        nc.scalar.activation(
                            x_pad[m][:, DC - 1 + nb * TB:
                                     DC - 1 + (nb + 1) * TB],
                            ps[:], AF.Identity, bias=bias_sb[:, m:m + 1])
                    else:
                        z = bkp.tile([128, TB], BF, tag=f"z{m - 3}",
                                     name=f"z{m - 3}")
                        nc.scalar.activation(z[:], ps[:], AF.Silu,
                                             bias=bias_sb[:, m:m + 1])
                        z_t[(nb, m - 3)] = z
                for j in range(NJ):
                    terms = []
                    for k in range(DC):
                        ak = cp.tile([128, TB], BF, tag=f"cm{k}", name=f"cm{k}")
                        nc.vector.tensor_scalar(
                            out=ak[:], in0=x_pad[j][:, nb * TB + k:
                                                    nb * TB + k + TB],
                            scalar1=convw_sb[:, j * DC + k:j * DC + k + 1],
                            scalar2=None, op0=OP.mult)
                        terms.append(ak)
                    s01 = cp.tile([128, TB], BF, tag="s01")
                    nc.gpsimd.tensor_tensor(out=s01[:], in0=terms[0][:],
                                            in1=terms[1][:], op=OP.add)
                    s23 = cp.tile([128, TB], BF, tag="s23")
                    nc.gpsimd.tensor_tensor(out=s23[:], in0=terms[2][:],
                                            in1=terms[3][:], op=OP.add)
                    st = cp.tile([128, TB], BF, tag="st")
                    nc.gpsimd.tensor_tensor(out=st[:], in0=s01[:],
                                            in1=s23[:], op=OP.add)
                    xc = bkp.tile([128, TB], BF, tag=f"xc{j}", name=f"xc{j}")
                    nc.scalar.activation(xc[:], st[:], AF.Silu,
                                         bias=convb_sb[:, j:j + 1])
                    xc_t[(nb, j)] = xc
                ps = xps.tile([80, TB], F, name="ps_xp")
                for j in range(NJ):
                    nc.tensor.matmul(ps[:], wxT_sb[j][:], xc_t[(nb, j)][:],
                                     start=(j == 0), stop=(j == NJ - 1))
                pp = nbp.tile([80, TB], F, tag="pp", name="pp")
                nc.scalar.copy(pp[:], ps[:])
                nc.sync.dma_start(proj_part[bass.ts(nb, 80), :], pp[:])
                if single:
                    nc.sync.dma_start(proj_sum[bass.ts(nb, 80), :],
                                      proj_part[bass.ts(nb, 80), :])
                else:
                    nc.gpsimd.collective_compute(
                        "AllReduce", OP.add, replica_groups=GROUPS,
                        ins=[proj_part[nb * 80:(nb + 1) * 80, :].opt()],
                        outs=[proj_sum[nb * 80:(nb + 1) * 80, :].opt()])
                bc32 = nbp.tile([2 * DS, TB], F, tag="bc32", name="bc32")
                nc.sync.dma_start(bc32[:],
                                  proj_sum[nb * 80 + DTR:
                                           nb * 80 + DTR + 2 * DS, :])
                bcb = nbp.tile([2 * DS, TB], BF, tag="bcb", name="bcb")
                nc.scalar.copy(bcb[:], bc32[:])
                nc.sync.dma_start(bc_bf[:, tsl], bcb[:])
                dtT = dtp.tile([DTR, TB], FR, tag="dtT", name="dtT")
                nc.sync.dma_start(dtT[:],
                                  proj_sum[nb * 80:nb * 80 + DTR, :]
                                  .bitcast(FR))
                ets = []
                for j in range(NJ):
                    psd = dps.tile([128, TB], F, name="psd")
                    nc.tensor.matmul(psd[:], wdtT_sb[:, bass.ts(j, 128)],
                                     dtT[:], start=True, stop=True)
                    et = nbp.tile([128, TB], F, tag=f"et{j}", name=f"et{j}")
                    nc.scalar.activation(et[:], psd[:], AF.Exp,
                                         bias=bdt_sb[:, j:j + 1])
                    ets.append(et)
                for j in range(NJ):
                    dl = bkp.tile([128, TB], BF, tag=f"dl{j}", name=f"dl{j}")
                    nc.scalar.activation(dl[:], ets[j][:], AF.Ln, bias=1.0)
                    dl_t[(nb, j)] = dl
                    ub = up.tile([128, TB], BF, tag=f"ub{j}", name=f"ub{j}")
                    nc.vector.tensor_mul(ub[:], dl[:], xc_t[(nb, j)][:])
                    ub_t[(nb, j)] = ub

            dA_t, dbx_t, rep_t = {}, {}, {}

            def emit_scan_pre(nb, half):
                """Broadcast B/C rows, compute dA (Act) and dBx (Pool)."""
                tsl = slice(nb * TB, (nb + 1) * TB)
                bc_rep = bp.tile([128, 2 * NH * TB], BF, tag="bcr", name="bcr")
                nc.sync.dma_start(
                    bc_rep[:, 0:NH * TB].rearrange("p (n t) -> p n t", n=NH),
                    bc_bf[half * NH:(half + 1) * NH, tsl]
                    .unsqueeze(0).broadcast_to([128, NH, TB]))
                nc.sync.dma_start(
                    bc_rep[:, NH * TB:2 * NH * TB]
                    .rearrange("p (n t) -> p n t", n=NH),
                    bc_bf[DS + half * NH:DS + (half + 1) * NH, tsl]
                    .unsqueeze(0).broadcast_to([128, NH, TB]))
                rep_t[(nb, half)] = bc_rep
                b_rep = bc_rep[:, 0:NH * TB]
                for j in range(NJ):
                    dA = adp.tile([128, NH * TB], BF, tag="dA", name="dA")
                    for n in range(NH):
                        nc.scalar.activation(
                            dA[:, bass.ts(n, TB)], dl_t[(nb, j)][:], AF.Exp,
                            scale=acol_sb[:, j * DS + half * NH + n:
                                          j * DS + half * NH + n + 1])
                    dA_t[(nb, half, j)] = dA
                    dbx = dbp.tile([128, NH * TB], BF, tag="dbx", name="dbx")
                    for hv in range(2):
                        sl4 = slice(hv * NH // 2 * TB,
                                    (hv + 1) * NH // 2 * TB)
                        nc.gpsimd.tensor_tensor(
                            out=dbx[:, sl4].rearrange(
                                "p (n t) -> p n t", n=NH // 2),
                            in0=ub_t[(nb, j)][:].unsqueeze(1)
                            .broadcast_to([128, NH // 2, TB]),
                            in1=b_rep[:, sl4].rearrange(
                                "p (n t) -> p n t", n=NH // 2),
                            op=OP.mult)
                    dbx_t[(nb, half, j)] = dbx

            def emit_scan_consume(nb, half):
                """DVE: scans + carry + y = sum_n h*C."""
                c_rep = rep_t[(nb, half)][:, NH * TB:2 * NH * TB]
                for j in range(NJ):
                    dA = dA_t.pop((nb, half, j))
                    dbx = dbx_t.pop((nb, half, j))
                    h = hp.tile([128, NH * TB], BF, tag="h", name="h")
                    for n in range(NH):
                        nc.vector.tensor_tensor_scan(
                            out=h[:, bass.ts(n, TB)],
                            data0=dA[:, bass.ts(n, TB)],
                            data1=dbx[:, bass.ts(n, TB)],
                            initial=carry[j][:, half * NH + n:
                                             half * NH + n + 1],
                            op0=OP.mult, op1=OP.add)
                    nc.vector.tensor_copy(
                        carry[j][:, half * NH:(half + 1) * NH],
                        h[:].rearrange("p (n t) -> p n t",
                                       n=NH)[:, :, TB - 1])
                    yp = ypp.tile([128, NH * TB], BF, tag="yp", name="yp")
                    nc.vector.tensor_mul(yp[:], h[:], c_rep)
                    t2 = ytp.tile([128, 4 * TB], BF, tag="t2", name="t2")
                    nc.vector.tensor_add(t2[:], yp[:, 0:4 * TB],
                                         yp[:, 4 * TB:8 * TB])
                    t3 = ytp.tile([128, 2 * TB], BF, tag="t3", name="t3")
                    nc.vector.tensor_add(t3[:], t2[:, 0:2 * TB],
                                         t2[:, 2 * TB:4 * TB])
                    if half == 0:
                        y = bkp.tile([128, TB], BF, tag=f"y{j}", name=f"y{j}")
                        nc.vector.tensor_add(y[:], t3[:, 0:TB],
                                             t3[:, TB:2 * TB])
                        y_t[(nb, j)] = y
                    else:
                        yt = ytp.tile([128, TB], BF, tag="yt", name="yt")
                        nc.vector.tensor_add(yt[:], t3[:, 0:TB],
                                             t3[:, TB:2 * TB])
                        nc.vector.tensor_add(y_t[(nb, j)][:],
                                             y_t[(nb, j)][:], yt[:])

            def emit_fin(nb):
                for j in range(NJ):
                    t0 = fp.tile([128, TB], BF, tag="t0", name="t0")
                    nc.vector.tensor_scalar(out=t0[:], in0=xc_t[(nb, j)][:],
                                            scalar1=dskip_sb[:, j:j + 1],
                                            scalar2=None, op0=OP.mult)
                    t1 = fp.tile([128, TB], BF, tag="t1", name="t1")
                    nc.vector.tensor_add(t1[:], t0[:], y_t[(nb, j)][:])
                    nc.vector.tensor_mul(y_t[(nb, j)][:], t1[:],
                                         z_t[(nb, j)][:])

            emit_front(0)
            emit_scan_pre(0, 0)
            for nb in range(4):
                emit_scan_pre(nb, 1)
                emit_scan_consume(nb, 0)
                if nb + 1 < 4:
                    emit_front(nb + 1)
                    emit_scan_pre(nb + 1, 0)
                emit_scan_consume(nb, 1)
                emit_fin(nb)
                if nb > 0:
                    emit_outproj(nb - 1)

            emit_outproj(3)

            # ---- ReduceScatter + output tail ----
            for m in range(6):
                if single:
                    nc.sync.dma_start(op_rs[bass.ts(m, 128), :],
                                      op_part[m * 512:m * 512 + 128, :])
                else:
                    nc.gpsimd.collective_compute(
                        "ReduceScatter", OP.add, replica_groups=GROUPS,
                        ins=[op_part[m * 512:(m + 1) * 512, :].opt()],
                        outs=[op_rs[bass.ts(m, 128), :].opt()])
                nc.sync.dma_start(hid_out[bass.ts(m, 128), :],
                                  op_rs[bass.ts(m, 128), :])

    nc.compile()
    _CACHE[key] = nc
    return nc


def _prep_inputs(inp):
    gamma, beta = inp["ln_gamma"], inp["ln_beta"]
    W_in = inp["W_in"]
    W_in_f = W_in * gamma[None, :]
    bias_full = W_in @ beta            # [2*DI]
    A = -np.exp(inp["A_log"])          # [DI, DS]
    bf = ml_dtypes.bfloat16

    in_maps = []
    for c in range(NCORES):
        b, j = c // 4, c % 4
        S = slice(j * SL, (j + 1) * SL)
        rows = np.r_[j * SL:(j + 1) * SL, DI + j * SL:DI + (j + 1) * SL]
        m = {
            "x_b": inp["x"][b],
            "res_x": inp["x"][b, j * TOK:(j + 1) * TOK],
            "res_in": inp["residual"][b, j * TOK:(j + 1) * TOK],
            "W_inT": np.ascontiguousarray(W_in_f[rows].T).astype(bf),
            "bias_in": np.ascontiguousarray(bias_full[rows]),
            "WxT": np.ascontiguousarray(inp["W_xproj"][:, S].T).astype(bf),
            "WdtT": np.ascontiguousarray(inp["W_dt"][S].T),
            "bdt": np.ascontiguousarray(inp["b_dt"][S]),
            "Acols": np.ascontiguousarray(A[S]),
            "convw": np.ascontiguousarray(inp["conv_w"][S]),
            "convb": np.ascontiguousarray(inp["conv_b"][S]),
            "Dskip": np.ascontiguousarray(inp["D_skip"][S]),
            "WoT": np.ascontiguousarray(inp["W_out"][:, S].T).astype(bf),
        }
        in_maps.append(m)
    return in_maps


def _assemble(results):
    hidden = np.empty((B, L, DM), np.float32)
    residual = np.empty((B, L, DM), np.float32)
    for c in range(NCORES):
        b, j = c // 4, c % 4
        r = results[c]
        hidden[b, j * TOK:(j + 1) * TOK] = r["hid_out"].T
        residual[b, j * TOK:(j + 1) * TOK] = r["res_out"]
    return hidden, residual


def kernel(**inputs):
    inp = {k: np.ascontiguousarray(np.asarray(v, dtype=np.float32))
           for k, v in inputs.items()}
    nc = _build()
    in_maps = _prep_inputs(inp)
    res = run_bass_kernel_spmd(nc, in_maps, list(range(NCORES)))
    return _assemble(res.results)
